# revision 38
# baseline (speedup 1.0000x reference)
import sys

import numpy as np
from ml_dtypes import bfloat16

sys.path.insert(0, "/opt/trn_rl_repo")

TRACE = False
LAST = {}
_cache = {}

SPARSITY = 0.5

# even columns first, then odd: makes the stride-2 convs read contiguously
_XPERM = np.r_[1:64:2, 0:64:2]


def _tf32(a):
    b = np.ascontiguousarray(np.asarray(a, np.float32))
    u = b.view(np.uint32).copy()
    u += np.uint32(0x0FFF) + ((u >> np.uint32(13)) & np.uint32(1))
    u &= np.uint32(0xFFFFE000)
    return u.view(np.float32)


def _masked(w, s):
    sa = np.abs(np.asarray(s, np.float32)).ravel()
    j = int((1.0 - SPARSITY) * sa.size)
    thr = np.partition(sa, j)[j]
    m = (np.abs(np.asarray(s, np.float32)) >= thr).astype(np.float32)
    return (np.asarray(w, np.float32) * m).astype(np.float32)


def _prep(inputs):
    w1m = _masked(inputs["w1"], inputs["s1"])  # [128,3,3,3]
    w2m = _masked(inputs["w2"], inputs["s2"])  # [256,128,3,3]
    w3m = _masked(inputs["w3"], inputs["s3"])  # [512,256,3,3]
    fw1m = _masked(inputs["fw1"], inputs["fs1"])  # [1024,512]
    fw2m = _masked(inputs["fw2"], inputs["fs2"])  # [10,1024]

    c = np.ascontiguousarray
    # conv1 as single K=27 matmul, K padded to 32 and replicated 4x across
    # partition groups so rhs tiles at base partitions 0/32/64/96 line up
    w1t = np.zeros((64, 128), np.float32)
    w1t[:27] = w1m.transpose(1, 2, 3, 0).reshape(27, 128)
    w1t[32:59] = w1t[:27]
    w2t = c(w2m.transpose(1, 2, 3, 0).reshape(128, 9 * 256))
    # mt-major: [k2, mt, kt*1152 + g*128 + o]
    w3t = c(
        w3m.reshape(4, 128, 2, 128, 3, 3)
        .transpose(3, 0, 2, 4, 5, 1)
        .reshape(128, 4, 2304)
    )
    # global-avg-pool 1/256 folded into fw1
    fw1t = c((fw1m.T.reshape(4, 128, 1024).transpose(1, 0, 2) / 256.0).astype(np.float32))
    fw2t = c(fw2m.T.reshape(8, 128, 10).transpose(1, 0, 2))

    weights = {
        "w1t": w1t.astype(bfloat16),
        "w2t": w2t.astype(bfloat16),
        "w3t": w3t.astype(bfloat16),
        "fw1t": fw1t.astype(bfloat16),
        "fw2t": fw2t.astype(bfloat16),
        "bias": np.concatenate(
            [
                np.asarray(inputs["b1"], np.float32).reshape(128, 1),
                np.asarray(inputs["b2"], np.float32).reshape(2, 128).T,
                np.asarray(inputs["b3"], np.float32).reshape(4, 128).T,
            ],
            axis=1,
        ),
        "fb1": np.concatenate(
            [np.asarray(inputs["fb1"], np.float32).reshape(1, 1024),
             np.asarray(inputs["fb2"], np.float32).reshape(1, 10),
             np.zeros((1, 6), np.float32)],
            axis=1,
        ).astype(bfloat16),
        "ident": np.eye(8, dtype=np.float32).astype(bfloat16),
    }
    xpad = np.zeros((64, 3, 66, 66), np.float32)
    xpad[:, :, 1:65, 1:65] = np.asarray(inputs["x"], np.float32)
    # im2col over (ch,ky,kx): x27[i, ch*9+ky*3+kx] = xpad[i, ch, ky:ky+64, kx:kx+64]
    x27 = np.empty((64, 27, 64, 64), np.float32)
    for ch in range(3):
        for ky in range(3):
            for kx in range(3):
                x27[:, ch * 9 + ky * 3 + kx] = xpad[:, ch, ky : ky + 64, kx : kx + 64]
    # pack for full-width DMA + phase-split columns:
    # xim32[i, half, 32*nt + k, r, px] = x27[i, k, 32*half + 8*nt + r, XPERM[px]]
    xr = x27[:, :, :, _XPERM].reshape(64, 27, 2, 4, 8, 64)  # [i,k,half,nt,r,px]
    # partition p = 32*a + k holds nt = 2*b + a of half h at free slot 2h+b:
    # matmul rhs bases stay at 0/32, one whole-image DMA per image.
    xim32 = np.zeros((64, 2, 32, 4, 8, 64), np.float32)  # [i,a,k,2h+b,r,px]
    for a in range(2):
        for h in range(2):
            for b in range(2):
                xim32[:, a, :27, 2 * h + b] = xr[:, :, h, 2 * b + a]
    xim32 = c(xim32.reshape(64, 64, 4, 8, 64).astype(bfloat16))
    return xim32, weights


def _build():
    import concourse.bacc as bacc
    import concourse.mybir as mybir
    import concourse.tile as tile

    FP = mybir.dt.float32
    FR = mybir.dt.float32r
    BF = mybir.dt.bfloat16
    RELU = mybir.ActivationFunctionType.Relu
    ADD = mybir.AluOpType.add
    MAX = mybir.AluOpType.max

    nc = bacc.Bacc("TRN2", target_bir_lowering=False, debug=False)

    xpad_d = nc.dram_tensor("xpad", [8, 64, 4, 8, 64], BF, kind="ExternalInput")
    w1t_d = nc.dram_tensor("w1t", [64, 128], BF, kind="ExternalInput")
    w2t_d = nc.dram_tensor("w2t", [128, 2304], BF, kind="ExternalInput")
    w3t_d = nc.dram_tensor("w3t", [128, 4, 2304], BF, kind="ExternalInput")
    fw1t_d = nc.dram_tensor("fw1t", [128, 4, 1024], BF, kind="ExternalInput")
    fw2t_d = nc.dram_tensor("fw2t", [128, 8, 10], BF, kind="ExternalInput")
    bias_d = nc.dram_tensor("bias", [128, 7], FP, kind="ExternalInput")
    fb1_d = nc.dram_tensor("fb1", [1, 1040], BF, kind="ExternalInput")
    ident_d = nc.dram_tensor("ident", [8, 8], BF, kind="ExternalInput")
    outT_d = nc.dram_tensor("outT", [10, 8], FP, kind="ExternalOutput")

    with tile.TileContext(nc) as tc:
        with tc.tile_pool(name="consts", bufs=1) as consts, \
             tc.tile_pool(name="xim_p", bufs=2) as xim_p, \
             tc.tile_pool(name="act_p", bufs=1) as act_p, \
             tc.tile_pool(name="h3_p", bufs=2) as h3_p, \
             tc.tile_pool(name="ps1_p", bufs=4, space="PSUM") as ps1_p, \
             tc.tile_pool(name="ps23_p", bufs=2, space="PSUM") as ps23_p, \
             tc.tile_pool(name="ps3_p", bufs=2, space="PSUM") as ps3_p:

            xims = {}

            def load_xim(img):
                t = xim_p.tile([64, 4, 8, 64], BF, name="ximg")
                nc.sync.dma_start(out=t[:, :, :, :], in_=xpad_d[img, :, :, :, :])
                xims[img] = t

            # DMA issue order = global transfer order: descriptor generation is
            # serialized (~0.7us each) on the Sync queue, so the tensors the
            # first matmuls need go first. Image 0 streams as two halves.
            xim00 = xim_p.tile([64, 2, 8, 64], BF, name="ximg")
            nc.sync.dma_start(out=xim00[:, :, :, :], in_=xpad_d[0, :, 0:2, :, :])
            w1t = consts.tile([64, 128], BF)
            nc.sync.dma_start(out=w1t[:, :], in_=w1t_d[:, :])
            xim01 = xim_p.tile([64, 2, 8, 64], BF, name="ximg")
            nc.sync.dma_start(out=xim01[:, :, :, :], in_=xpad_d[0, :, 2:4, :, :])
            xims[(0, 0)], xims[(0, 1)] = xim00, xim01
            bias = consts.tile([128, 7], FP)
            nc.sync.dma_start(out=bias[:, :], in_=bias_d[:, :])
            w2t = consts.tile([128, 2304], BF)
            nc.sync.dma_start(out=w2t[:, :], in_=w2t_d[:, :])
            load_xim(1)
            w3all = consts.tile([128, 4, 2304], BF)
            nc.sync.dma_start(out=w3all[:, :, :], in_=w3t_d[:, :, :])
            b1sb = bias[:, 0:1]
            b2sb = bias[:, 1:3]
            b3sb = bias[:, 3:7]

            # PE p-state warm-up: ramp runs on wall time since first dispatch,
            # so a burst of throwaway matmuls during the DMA wait gets the
            # engine to speed before conv1 starts.
            warm = consts.tile([32, 8, 64], BF)
            nc.vector.memset(warm[:, :, :], 0.0)
            ones = consts.tile([1, 8], BF)
            nc.vector.memset(ones[:, :], 1.0)
            for _ in range(4):
                wps = ps1_p.tile([128, 8, 64], FP, name="ps_c1")
                nc.tensor.matmul(
                    out=wps[0:64, :, :], lhsT=warm[:, 0, :], rhs=warm[:, :, :],
                    start=True, stop=True,
                )

            # h1 phase-split in one tile: cols 0:33 = even input cols
            # (0,2,..,64), cols 33:66 = odd (1,3,..,65). With the odds-first
            # pixel order from the host, a conv1 PSUM tile drains to
            # h1eo[:, rows, 1:65] in a single contiguous op.
            h1eo_a = act_p.tile([128, 66, 66], BF)
            h1eo_b = act_p.tile([128, 66, 66], BF)
            # h2 phase-split: h2e = cols 0,2,..,32 (17), h2o = cols 1,..,33 (17)
            h2e_a = act_p.tile([128, 2, 2, 34, 17], BF)
            h2o_a = act_p.tile([128, 2, 2, 34, 17], BF)
            h2e_b = act_p.tile([128, 2, 2, 34, 17], BF)
            h2o_b = act_p.tile([128, 2, 2, 34, 17], BF)
            hpool = act_p.tile([128, 4, 8], BF)
            z1 = act_p.tile([8, 1024], BF)
            z1T = act_p.tile([128, 8, 8], BF)
            y_sb = act_p.tile([10, 8], FP)

            # Border-only zeroing: interiors are fully overwritten every image,
            # borders stay zero for the kernel's lifetime.
            for h1eo in (h1eo_a, h1eo_b):
                nc.vector.memset(h1eo[:, 0, :], 0.0)
                nc.vector.memset(h1eo[:, 65, :], 0.0)
                nc.vector.memset(h1eo[:, 1:65, 0], 0.0)
                nc.vector.memset(h1eo[:, 1:65, 65], 0.0)
            for h2e, h2o in ((h2e_a, h2o_a), (h2e_b, h2o_b)):
                for m in range(2):
                    for i in range(2):
                        nc.vector.memset(h2e[:, m, i, 0, :], 0.0)
                        nc.vector.memset(h2e[:, m, i, 33, :], 0.0)
                        nc.vector.memset(h2e[:, m, i, 1:33, 0], 0.0)
                        nc.vector.memset(h2o[:, m, i, 0, :], 0.0)
                        nc.vector.memset(h2o[:, m, i, 33, :], 0.0)
                        nc.vector.memset(h2o[:, m, i, 1:33, 16], 0.0)

            h1pads = [h1eo_a, h1eo_b]
            h2pads = [(h2e_a, h2o_a), (h2e_b, h2o_b)]

            def drain(eng, out, ps, bias):
                if eng == 0:
                    nc.scalar.activation(out=out, in_=ps, func=RELU, bias=bias)
                else:
                    nc.vector.tensor_scalar(
                        out=out, in0=ps, scalar1=bias, scalar2=0.0,
                        op0=ADD, op1=MAX,
                    )

            def conv1_half(img, h1eo, half):
                if img == 0:
                    xim = xims.pop((0, half))
                    slot = 0
                else:
                    xim = xims.pop(img) if half == 1 else xims[img]
                    slot = 2 * half
                for nt in range(4):
                    ntg = 4 * half + nt
                    ps = ps1_p.tile([128, 8, 64], FP, name="ps_c1")
                    nc.tensor.matmul(
                        out=ps[:, :, :],
                        lhsT=w1t[32 * (nt % 2) : 32 * (nt % 2) + 32, :],
                        rhs=xim[32 * (nt % 2) : 32 * (nt % 2) + 32, slot + nt // 2, :, :],
                        start=True,
                        stop=True,
                    )
                    # odds-first pixel order: ps col j -> h1eo col j+1;
                    # split across both engines to halve the PSUM WAR latency
                    r0 = 1 + 8 * ntg
                    drain(ntg % 2, h1eo[:, r0 : r0 + 8, 1:33],
                          ps[:, :, 0:32], b1sb[:, 0:1])
                    drain(1 - ntg % 2, h1eo[:, r0 : r0 + 8, 33:65],
                          ps[:, :, 32:64], b1sb[:, 0:1])

            def conv2_half(img, h1eo, h2pair, islot, nh):
                h2e, h2o = h2pair
                # g-outer / m-inner: each rhs slice feeds both m-groups
                # back-to-back, amortizing the SBUF segment fetches
                pss = [ps23_p.tile([128, 16, 32], FP, name="ps_c2") for _ in range(2)]
                for g in range(9):
                    ky, kx = g // 3, g % 3
                    c0 = 0 if kx == 0 else 33 if kx == 1 else 1
                    r0 = 32 * nh + ky
                    for m in range(2):
                        nc.tensor.matmul(
                            out=pss[m][:, :, :],
                            lhsT=w2t[:, 256 * g + 128 * m : 256 * g + 128 * m + 128],
                            rhs=h1eo[:, r0 : r0 + 32 : 2, c0 : c0 + 32],
                            start=(g == 0),
                            stop=(g == 8),
                        )
                # out x' even -> h2 odd cols -> h2o[0:16]; odd -> h2e[1:17]
                r0 = 1 + 16 * nh
                for m in range(2):
                    drain(0 if m == 0 else 1, h2o[:, m, islot, r0 : r0 + 16, 0:16],
                          pss[m][:, :, 0:32:2], b2sb[:, m : m + 1])
                    drain(1 if m == 0 else 0, h2e[:, m, islot, r0 : r0 + 16, 1:17],
                          pss[m][:, :, 1:32:2], b2sb[:, m : m + 1])

            def conv3(pair, h2pair, fc1_hook=None):
                h2e, h2o = h2pair
                for mt in range(4):
                    ps = ps3_p.tile([128, 2, 16, 16], FP, name="ps_c3")
                    n = 0
                    for kt in range(2):
                        for g in range(9):
                            ky, kx = g // 3, g % 3
                            hsrc, c0 = (
                                (h2e, 0) if kx == 0 else (h2o, 0) if kx == 1 else (h2e, 1)
                            )
                            nc.tensor.matmul(
                                out=ps[:, :, :, :],
                                lhsT=w3all[:, mt, 1152 * kt + 128 * g : 1152 * kt + 128 * g + 128],
                                rhs=hsrc[:, kt, :, ky : ky + 32 : 2, c0 : c0 + 16],
                                start=(n == 0),
                                stop=(n == 17),
                            )
                            n += 1
                    h3 = h3_p.tile([128, 2, 16, 16], FP, name="h3scr")
                    # accumulation runs in fp32 internally; only the final
                    # write is f32r-rounded (it feeds a tf32 matmul anyway)
                    with nc.allow_low_precision(reason="pool feeds f32r matmul"):
                        nc.scalar.activation(
                            out=h3[:, 0, :, :],
                            in_=ps[:, 0, :, :],
                            func=RELU,
                            bias=b3sb[:, mt : mt + 1],
                            accum_out=hpool[:, mt, 2 * pair : 2 * pair + 1],
                        )
                        nc.vector.tensor_scalar(
                            out=h3[:, 1, :, :], in0=ps[:, 1, :, :],
                            scalar1=b3sb[:, mt : mt + 1], scalar2=0.0,
                            op0=ADD, op1=MAX,
                        )
                        nc.vector.tensor_reduce(
                            out=hpool[:, mt, 2 * pair + 1 : 2 * pair + 2],
                            in_=h3[:, 1, :, :],
                            axis=mybir.AxisListType.XY,
                            op=ADD,
                        )
                    if fc1_hook is not None:
                        fc1_hook(mt)

            # conv3(pair p) is sandwiched between conv1 and conv2 of image
            # 2p+2: conv1's matmuls cover the latency of the last conv2
            # drains conv3 depends on, and conv3's long stretch covers
            # conv1's drains that conv2 depends on.
            fw1t = fw2t = fb1row = ident = None
            for img in range(8):
                pair, i = divmod(img, 2)
                h1pair = h1pads[img % 2]
                h2pair = h2pads[pair % 2]
                for half in range(2):
                    conv1_half(img, h1pair, half)
                    if half == 0 and img + 2 < 8:
                        load_xim(img + 2)
                if i == 0 and pair >= 1:
                    conv3(pair - 1, h2pads[(pair - 1) % 2])
                if img == 1:
                    fw1t = consts.tile([128, 4, 1024], BF)
                    nc.sync.dma_start(out=fw1t[:, :, :], in_=fw1t_d[:, :, :])
                    fw2t = consts.tile([128, 8, 10], BF)
                    nc.sync.dma_start(out=fw2t[:, :, :], in_=fw2t_d[:, :, :])
                    fb1row = consts.tile([1, 1040], BF)
                    nc.sync.dma_start(out=fb1row[:, :], in_=fb1_d[:, :])
                    ident = consts.tile([8, 8], BF)
                    nc.sync.dma_start(out=ident[:, :], in_=ident_d[:, :])
                for half in range(2):
                    conv2_half(img, h1pair, h2pair, i, half)

            # FC1 with batch on partitions: psf[b, j] = fb1[j] + sum_kt
            # hpool[:, kt, b]^T @ fw1t[:, kt, j]. One 512-wide matmul per
            # (kt, half): 4 LDWEIGHTS of hpool instead of 32 of fw1t, so the
            # PE isn't weight-load-bound. Bias lands first via a K=1 matmul
            # (all-ones lhsT). Chunk kt is issued one mt-group late inside
            # conv3(pair 3) so its hpool dependency is long satisfied and
            # only kt=3 remains on the serial tail.
            psfA = ps1_p.tile([8, 512], FP, name="ps_c1")
            psfB = ps1_p.tile([8, 512], FP, name="ps_c1")
            nc.tensor.matmul(
                out=psfA[:, :], lhsT=ones[0:1, :], rhs=fb1row[0:1, 0:512],
                start=True, stop=False,
            )
            nc.tensor.matmul(
                out=psfB[:, :], lhsT=ones[0:1, :], rhs=fb1row[0:1, 512:1024],
                start=True, stop=False,
            )

            def fc1_chunk(kt):
                nc.tensor.matmul(
                    out=psfA[:, :], lhsT=hpool[:, kt, :], rhs=fw1t[:, kt, 0:512],
                    start=False, stop=(kt == 3),
                )
                nc.tensor.matmul(
                    out=psfB[:, :], lhsT=hpool[:, kt, :], rhs=fw1t[:, kt, 512:1024],
                    start=False, stop=(kt == 3),
                )

            def fc1_hook(mt):
                if mt >= 1:
                    fc1_chunk(mt - 1)

            conv3(3, h2pads[1], fc1_hook=fc1_hook)
            fc1_chunk(3)

            # relu in 256-col chunks alternating engines so the first
            # transpose starts ~350ns after psfA stops instead of 1.2us
            for q in range(4):
                ps_src = psfA if q < 2 else psfB
                off = 256 * (q % 2)
                if q % 2 == 0:
                    nc.scalar.activation(
                        out=z1[:, 256 * q : 256 * q + 256],
                        in_=ps_src[:, off : off + 256], func=RELU,
                    )
                else:
                    nc.vector.tensor_scalar(
                        out=z1[:, 256 * q : 256 * q + 256],
                        in0=ps_src[:, off : off + 256],
                        scalar1=0.0, scalar2=0.0, op0=ADD, op1=MAX,
                    )

            # z1 [8, 1024] -> z1T [128, 8, 8] via PE transpose (identity rhs),
            # drained by relu (idempotent) alternating engines; FC2 accumulates
            # over the 8 column chunks.
            psf2 = ps1_p.tile([128, 8], FP, name="ps_c1")
            nc.tensor.matmul(
                out=psf2[0:10, :], lhsT=fb1row[0:1, 1024:1034], rhs=ones[0:1, :],
                start=True, stop=False,
            )
            for c in range(8):
                zps = (ps23_p if c % 2 == 0 else ps3_p).tile(
                    [128, 8], FP, name="ps_c2" if c % 2 == 0 else "ps_c3"
                )
                nc.tensor.matmul(
                    out=zps[:, :], lhsT=z1[:, 128 * c : 128 * c + 128],
                    rhs=ident[0:8, 0:8], start=True, stop=True,
                )
                if c % 2 == 0:
                    nc.scalar.activation(out=z1T[:, c, :], in_=zps[:, :], func=RELU)
                else:
                    nc.vector.tensor_scalar(
                        out=z1T[:, c, :], in0=zps[:, :], scalar1=0.0, scalar2=0.0,
                        op0=ADD, op1=MAX,
                    )
                nc.tensor.matmul(
                    out=psf2[0:10, :],
                    lhsT=fw2t[:, c, :],
                    rhs=z1T[:, c, :],
                    start=False,
                    stop=(c == 7),
                )
            nc.scalar.activation(out=y_sb[:, :], in_=psf2[0:10, :], func=mybir.ActivationFunctionType.Copy)
            nc.sync.dma_start(out=outT_d[:, :], in_=y_sb[:, :])

    nc.compile()
    return nc


def _get_nc():
    if "nc" not in _cache:
        _cache["nc"] = _build()
    return _cache["nc"]


def kernel(**inputs):
    from concourse import bass_utils

    nc = _get_nc()
    xpad, weights = _prep(inputs)
    in_maps = [
        dict(weights, xpad=np.ascontiguousarray(xpad[8 * c : 8 * c + 8]))
        for c in range(8)
    ]
    res = bass_utils.run_bass_kernel_spmd(
        nc, in_maps, core_ids=list(range(8)), trace=TRACE
    )
    LAST["exec_time_ns"] = getattr(res, "exec_time_ns", None)
    LAST["profile_json"] = getattr(res, "profile_json", None)
    LAST["instructions_and_trace"] = getattr(res, "instructions_and_trace", None)
    out = np.concatenate([r["outT"].T for r in res.results], axis=0)
    return np.ascontiguousarray(out.astype(np.float32))


# revision 42
# speedup vs baseline: 1.0155x; 1.0155x over previous
import sys

import numpy as np
from ml_dtypes import bfloat16

sys.path.insert(0, "/opt/trn_rl_repo")

TRACE = False
LAST = {}
_cache = {}

SPARSITY = 0.5

# even columns first, then odd: makes the stride-2 convs read contiguously
_XPERM = np.r_[1:64:2, 0:64:2]


def _tf32(a):
    b = np.ascontiguousarray(np.asarray(a, np.float32))
    u = b.view(np.uint32).copy()
    u += np.uint32(0x0FFF) + ((u >> np.uint32(13)) & np.uint32(1))
    u &= np.uint32(0xFFFFE000)
    return u.view(np.float32)


def _masked(w, s):
    sa = np.abs(np.asarray(s, np.float32)).ravel()
    j = int((1.0 - SPARSITY) * sa.size)
    thr = np.partition(sa, j)[j]
    m = (np.abs(np.asarray(s, np.float32)) >= thr).astype(np.float32)
    return (np.asarray(w, np.float32) * m).astype(np.float32)


def _prep(inputs):
    w1m = _masked(inputs["w1"], inputs["s1"])  # [128,3,3,3]
    w2m = _masked(inputs["w2"], inputs["s2"])  # [256,128,3,3]
    w3m = _masked(inputs["w3"], inputs["s3"])  # [512,256,3,3]
    fw1m = _masked(inputs["fw1"], inputs["fs1"])  # [1024,512]
    fw2m = _masked(inputs["fw2"], inputs["fs2"])  # [10,1024]

    c = np.ascontiguousarray
    # conv1 as single K=27 matmul, K padded to 32 and replicated 4x across
    # partition groups so rhs tiles at base partitions 0/32/64/96 line up
    w1t = np.zeros((64, 128), np.float32)
    w1t[:27] = w1m.transpose(1, 2, 3, 0).reshape(27, 128)
    w1t[32:59] = w1t[:27]
    w2t = c(w2m.transpose(1, 2, 3, 0).reshape(128, 9 * 256))
    # mt-major: [k2, mt, kt*1152 + g*128 + o]
    w3t = c(
        w3m.reshape(4, 128, 2, 128, 3, 3)
        .transpose(3, 0, 2, 4, 5, 1)
        .reshape(128, 4, 2304)
    )
    # global-avg-pool 1/256 folded into fw1
    fw1t = c((fw1m.T.reshape(4, 128, 1024).transpose(1, 0, 2) / 256.0).astype(np.float32))
    fw2t = c(fw2m.T.reshape(8, 128, 10).transpose(1, 0, 2))

    weights = {
        "w1t": w1t.astype(bfloat16),
        "w2t": w2t.astype(bfloat16),
        "w3t": w3t.astype(bfloat16),
        "fw1t": fw1t.astype(bfloat16),
        "fw2t": fw2t.astype(bfloat16),
        "bias": np.concatenate(
            [
                np.asarray(inputs["b1"], np.float32).reshape(128, 1),
                np.asarray(inputs["b2"], np.float32).reshape(2, 128).T,
                np.asarray(inputs["b3"], np.float32).reshape(4, 128).T,
            ],
            axis=1,
        ),
        "fb1": np.concatenate(
            [np.asarray(inputs["fb1"], np.float32).reshape(1, 1024),
             np.asarray(inputs["fb2"], np.float32).reshape(1, 10),
             np.zeros((1, 6), np.float32)],
            axis=1,
        ).astype(bfloat16),
        "ident": np.eye(8, dtype=np.float32).astype(bfloat16),
    }
    xpad = np.zeros((64, 3, 66, 66), np.float32)
    xpad[:, :, 1:65, 1:65] = np.asarray(inputs["x"], np.float32)
    # im2col over (ch,ky,kx): x27[i, ch*9+ky*3+kx] = xpad[i, ch, ky:ky+64, kx:kx+64]
    x27 = np.empty((64, 27, 64, 64), np.float32)
    for ch in range(3):
        for ky in range(3):
            for kx in range(3):
                x27[:, ch * 9 + ky * 3 + kx] = xpad[:, ch, ky : ky + 64, kx : kx + 64]
    # pack for full-width DMA + phase-split columns:
    # xim32[i, half, 32*nt + k, r, px] = x27[i, k, 32*half + 8*nt + r, XPERM[px]]
    xr = x27[:, :, :, _XPERM].reshape(64, 27, 2, 4, 8, 64)  # [i,k,half,nt,r,px]
    # partition p = 32*a + k holds nt = 2*b + a of half h at free slot 2h+b:
    # matmul rhs bases stay at 0/32, one whole-image DMA per image.
    xim32 = np.zeros((64, 2, 32, 4, 8, 64), np.float32)  # [i,a,k,2h+b,r,px]
    for a in range(2):
        for h in range(2):
            for b in range(2):
                xim32[:, a, :27, 2 * h + b] = xr[:, :, h, 2 * b + a]
    xim32 = c(xim32.reshape(64, 64, 4, 8, 64).astype(bfloat16))
    return xim32, weights


def _build():
    import concourse.bacc as bacc
    import concourse.mybir as mybir
    import concourse.tile as tile

    FP = mybir.dt.float32
    FR = mybir.dt.float32r
    BF = mybir.dt.bfloat16
    RELU = mybir.ActivationFunctionType.Relu
    ADD = mybir.AluOpType.add
    MAX = mybir.AluOpType.max

    nc = bacc.Bacc("TRN2", target_bir_lowering=False, debug=False)

    xpad_d = nc.dram_tensor("xpad", [8, 64, 4, 8, 64], BF, kind="ExternalInput")
    w1t_d = nc.dram_tensor("w1t", [64, 128], BF, kind="ExternalInput")
    w2t_d = nc.dram_tensor("w2t", [128, 2304], BF, kind="ExternalInput")
    w3t_d = nc.dram_tensor("w3t", [128, 4, 2304], BF, kind="ExternalInput")
    fw1t_d = nc.dram_tensor("fw1t", [128, 4, 1024], BF, kind="ExternalInput")
    fw2t_d = nc.dram_tensor("fw2t", [128, 8, 10], BF, kind="ExternalInput")
    bias_d = nc.dram_tensor("bias", [128, 7], FP, kind="ExternalInput")
    fb1_d = nc.dram_tensor("fb1", [1, 1040], BF, kind="ExternalInput")
    ident_d = nc.dram_tensor("ident", [8, 8], BF, kind="ExternalInput")
    outT_d = nc.dram_tensor("outT", [10, 8], FP, kind="ExternalOutput")

    with tile.TileContext(nc) as tc:
        with tc.tile_pool(name="consts", bufs=1) as consts, \
             tc.tile_pool(name="xim_p", bufs=2) as xim_p, \
             tc.tile_pool(name="act_p", bufs=1) as act_p, \
             tc.tile_pool(name="h3_p", bufs=2) as h3_p, \
             tc.tile_pool(name="ps1_p", bufs=4, space="PSUM") as ps1_p, \
             tc.tile_pool(name="ps23_p", bufs=2, space="PSUM") as ps23_p, \
             tc.tile_pool(name="ps3_p", bufs=2, space="PSUM") as ps3_p:

            xims = {}

            def load_xim(img):
                t = xim_p.tile([64, 4, 8, 64], BF, name="ximg")
                nc.sync.dma_start(out=t[:, :, :, :], in_=xpad_d[img, :, :, :, :])
                xims[img] = t

            # DMA issue order = global transfer order: descriptor generation is
            # serialized (~0.7us each) on the Sync queue, so the tensors the
            # first matmuls need go first. Image 0 streams as two halves.
            xim00 = xim_p.tile([64, 2, 8, 64], BF, name="ximg")
            nc.sync.dma_start(out=xim00[:, :, :, :], in_=xpad_d[0, :, 0:2, :, :])
            w1t = consts.tile([64, 128], BF)
            nc.sync.dma_start(out=w1t[:, :], in_=w1t_d[:, :])
            xim01 = xim_p.tile([64, 2, 8, 64], BF, name="ximg")
            nc.sync.dma_start(out=xim01[:, :, :, :], in_=xpad_d[0, :, 2:4, :, :])
            xims[(0, 0)], xims[(0, 1)] = xim00, xim01
            bias = consts.tile([128, 7], FP)
            nc.sync.dma_start(out=bias[:, :], in_=bias_d[:, :])
            w2t = consts.tile([128, 2304], BF)
            nc.sync.dma_start(out=w2t[:, :], in_=w2t_d[:, :])
            load_xim(1)
            w3all = consts.tile([128, 4, 2304], BF)
            nc.sync.dma_start(out=w3all[:, :, :], in_=w3t_d[:, :, :])
            b1sb = bias[:, 0:1]
            b2sb = bias[:, 1:3]
            b3sb = bias[:, 3:7]

            # PE p-state warm-up: ramp runs on wall time since first dispatch,
            # so a burst of throwaway matmuls during the DMA wait gets the
            # engine to speed before conv1 starts.
            warm = consts.tile([32, 8, 64], BF)
            nc.vector.memset(warm[:, :, :], 0.0)
            ones = consts.tile([1, 8], BF)
            nc.vector.memset(ones[:, :], 1.0)
            for _ in range(4):
                wps = ps1_p.tile([128, 8, 64], FP, name="ps_c1")
                nc.tensor.matmul(
                    out=wps[0:64, :, :], lhsT=warm[:, 0, :], rhs=warm[:, :, :],
                    start=True, stop=True,
                )

            # h1 phase-split in one tile: cols 0:33 = even input cols
            # (0,2,..,64), cols 33:66 = odd (1,3,..,65). With the odds-first
            # pixel order from the host, a conv1 PSUM tile drains to
            # h1eo[:, rows, 1:65] in a single contiguous op.
            h1eo_a = act_p.tile([128, 66, 66], BF)
            h1eo_b = act_p.tile([128, 66, 66], BF)
            # h2 phase-split: h2e = cols 0,2,..,32 (17), h2o = cols 1,..,33 (17)
            h2e_a = act_p.tile([128, 2, 2, 34, 17], BF)
            h2o_a = act_p.tile([128, 2, 2, 34, 17], BF)
            h2e_b = act_p.tile([128, 2, 2, 34, 17], BF)
            h2o_b = act_p.tile([128, 2, 2, 34, 17], BF)
            hpool = act_p.tile([128, 4, 8], BF)
            z1 = act_p.tile([8, 1024], BF)
            z1T = act_p.tile([128, 8, 8], BF)
            y_sb = act_p.tile([10, 8], FP)

            # Border-only zeroing: interiors are fully overwritten every image,
            # borders stay zero for the kernel's lifetime.
            for h1eo in (h1eo_a, h1eo_b):
                nc.vector.memset(h1eo[:, 0, :], 0.0)
                nc.vector.memset(h1eo[:, 65, :], 0.0)
                nc.vector.memset(h1eo[:, 1:65, 0], 0.0)
                nc.vector.memset(h1eo[:, 1:65, 65], 0.0)
            for h2e, h2o in ((h2e_a, h2o_a), (h2e_b, h2o_b)):
                for m in range(2):
                    for i in range(2):
                        nc.vector.memset(h2e[:, m, i, 0, :], 0.0)
                        nc.vector.memset(h2e[:, m, i, 33, :], 0.0)
                        nc.vector.memset(h2e[:, m, i, 1:33, 0], 0.0)
                        nc.vector.memset(h2o[:, m, i, 0, :], 0.0)
                        nc.vector.memset(h2o[:, m, i, 33, :], 0.0)
                        nc.vector.memset(h2o[:, m, i, 1:33, 16], 0.0)

            h1pads = [h1eo_a, h1eo_b]
            h2pads = [(h2e_a, h2o_a), (h2e_b, h2o_b)]

            def drain(eng, out, ps, bias):
                if eng == 0:
                    nc.scalar.activation(out=out, in_=ps, func=RELU, bias=bias)
                else:
                    nc.vector.tensor_scalar(
                        out=out, in0=ps, scalar1=bias, scalar2=0.0,
                        op0=ADD, op1=MAX,
                    )

            def conv1_half(img, h1eo, half):
                if img == 0:
                    xim = xims.pop((0, half))
                    slot = 0
                else:
                    xim = xims.pop(img) if half == 1 else xims[img]
                    slot = 2 * half
                for nt in range(4):
                    ntg = 4 * half + nt
                    ps = ps1_p.tile([128, 8, 64], FP, name="ps_c1")
                    nc.tensor.matmul(
                        out=ps[:, :, :],
                        lhsT=w1t[32 * (nt % 2) : 32 * (nt % 2) + 32, :],
                        rhs=xim[32 * (nt % 2) : 32 * (nt % 2) + 32, slot + nt // 2, :, :],
                        start=True,
                        stop=True,
                    )
                    # odds-first pixel order: ps col j -> h1eo col j+1;
                    # split across both engines to halve the PSUM WAR latency
                    r0 = 1 + 8 * ntg
                    drain(ntg % 2, h1eo[:, r0 : r0 + 8, 1:33],
                          ps[:, :, 0:32], b1sb[:, 0:1])
                    drain(1 - ntg % 2, h1eo[:, r0 : r0 + 8, 33:65],
                          ps[:, :, 32:64], b1sb[:, 0:1])

            def conv2_half(img, h1eo, h2pair, islot, nh):
                h2e, h2o = h2pair
                # g-outer for rhs reuse; m=0 runs its last two passes early
                # so its drains overlap m=1's final matmuls and its PSUM slot
                # is free when the next half starts
                pss = [ps23_p.tile([128, 16, 32], FP, name="ps_c2") for _ in range(2)]
                # m1 trails m0 by 3 passes: the half opens m0-only (covering
                # the previous half's m1 drains) and closes m1-only (m0's
                # drains overlap), so neither PSUM slot is hot at a boundary
                order = [(0, 0), (1, 0), (2, 0)]
                for g in range(3, 9):
                    order += [(g, 0), (g - 3, 1)]
                order += [(6, 1), (7, 1), (8, 1)]

                def c2drain(m):
                    # out x' even -> h2 odd cols -> h2o[0:16]; odd -> h2e[1:17]
                    r0 = 1 + 16 * nh
                    drain(0 if m == 0 else 1, h2o[:, m, islot, r0 : r0 + 16, 0:16],
                          pss[m][:, :, 0:32:2], b2sb[:, m : m + 1])
                    drain(1 if m == 0 else 0, h2e[:, m, islot, r0 : r0 + 16, 1:17],
                          pss[m][:, :, 1:32:2], b2sb[:, m : m + 1])

                for g, m in order:
                    ky, kx = g // 3, g % 3
                    c0 = 0 if kx == 0 else 33 if kx == 1 else 1
                    r0 = 32 * nh + ky
                    nc.tensor.matmul(
                        out=pss[m][:, :, :],
                        lhsT=w2t[:, 256 * g + 128 * m : 256 * g + 128 * m + 128],
                        rhs=h1eo[:, r0 : r0 + 32 : 2, c0 : c0 + 32],
                        start=(g == 0),
                        stop=(g == 8),
                    )
                    if (g, m) == (8, 0):
                        c2drain(0)
                c2drain(1)

            def conv3(pair, h2pair, fc1_hook=None):
                h2e, h2o = h2pair
                for mt in range(4):
                    ps = ps3_p.tile([128, 2, 16, 16], FP, name="ps_c3")
                    n = 0
                    for kt in range(2):
                        for g in range(9):
                            ky, kx = g // 3, g % 3
                            hsrc, c0 = (
                                (h2e, 0) if kx == 0 else (h2o, 0) if kx == 1 else (h2e, 1)
                            )
                            nc.tensor.matmul(
                                out=ps[:, :, :, :],
                                lhsT=w3all[:, mt, 1152 * kt + 128 * g : 1152 * kt + 128 * g + 128],
                                rhs=hsrc[:, kt, :, ky : ky + 32 : 2, c0 : c0 + 16],
                                start=(n == 0),
                                stop=(n == 17),
                            )
                            n += 1
                    h3 = h3_p.tile([128, 2, 16, 16], FP, name="h3scr")
                    # accumulation runs in fp32 internally; only the final
                    # write is f32r-rounded (it feeds a tf32 matmul anyway)
                    with nc.allow_low_precision(reason="pool feeds f32r matmul"):
                        nc.scalar.activation(
                            out=h3[:, 0, :, :],
                            in_=ps[:, 0, :, :],
                            func=RELU,
                            bias=b3sb[:, mt : mt + 1],
                            accum_out=hpool[:, mt, 2 * pair : 2 * pair + 1],
                        )
                        nc.vector.tensor_scalar(
                            out=h3[:, 1, :, :], in0=ps[:, 1, :, :],
                            scalar1=b3sb[:, mt : mt + 1], scalar2=0.0,
                            op0=ADD, op1=MAX,
                        )
                        nc.vector.tensor_reduce(
                            out=hpool[:, mt, 2 * pair + 1 : 2 * pair + 2],
                            in_=h3[:, 1, :, :],
                            axis=mybir.AxisListType.XY,
                            op=ADD,
                        )
                    if fc1_hook is not None:
                        fc1_hook(mt)

            # conv3(pair p) is sandwiched between conv1 and conv2 of image
            # 2p+2: conv1's matmuls cover the latency of the last conv2
            # drains conv3 depends on, and conv3's long stretch covers
            # conv1's drains that conv2 depends on.
            fw1t = fw2t = fb1row = ident = None
            for img in range(8):
                pair, i = divmod(img, 2)
                h1pair = h1pads[img % 2]
                h2pair = h2pads[pair % 2]
                for half in range(2):
                    conv1_half(img, h1pair, half)
                    if half == 0 and img + 2 < 8:
                        load_xim(img + 2)
                if i == 0 and pair >= 1:
                    conv3(pair - 1, h2pads[(pair - 1) % 2])
                if img == 1:
                    fw1t = consts.tile([128, 4, 1024], BF)
                    nc.sync.dma_start(out=fw1t[:, :, :], in_=fw1t_d[:, :, :])
                    fw2t = consts.tile([128, 8, 10], BF)
                    nc.sync.dma_start(out=fw2t[:, :, :], in_=fw2t_d[:, :, :])
                    fb1row = consts.tile([1, 1040], BF)
                    nc.sync.dma_start(out=fb1row[:, :], in_=fb1_d[:, :])
                    ident = consts.tile([8, 8], BF)
                    nc.sync.dma_start(out=ident[:, :], in_=ident_d[:, :])
                for half in range(2):
                    conv2_half(img, h1pair, h2pair, i, half)

            # FC1 with batch on partitions: psf[b, j] = fb1[j] + sum_kt
            # hpool[:, kt, b]^T @ fw1t[:, kt, j]. One 512-wide matmul per
            # (kt, half): 4 LDWEIGHTS of hpool instead of 32 of fw1t, so the
            # PE isn't weight-load-bound. Bias lands first via a K=1 matmul
            # (all-ones lhsT). Chunk kt is issued one mt-group late inside
            # conv3(pair 3) so its hpool dependency is long satisfied and
            # only kt=3 remains on the serial tail.
            psfA = ps1_p.tile([8, 512], FP, name="ps_c1")
            psfB = ps1_p.tile([8, 512], FP, name="ps_c1")
            nc.tensor.matmul(
                out=psfA[:, :], lhsT=ones[0:1, :], rhs=fb1row[0:1, 0:512],
                start=True, stop=False,
            )
            nc.tensor.matmul(
                out=psfB[:, :], lhsT=ones[0:1, :], rhs=fb1row[0:1, 512:1024],
                start=True, stop=False,
            )

            def fc1_chunk(kt):
                nc.tensor.matmul(
                    out=psfA[:, :], lhsT=hpool[:, kt, :], rhs=fw1t[:, kt, 0:512],
                    start=False, stop=(kt == 3),
                )
                nc.tensor.matmul(
                    out=psfB[:, :], lhsT=hpool[:, kt, :], rhs=fw1t[:, kt, 512:1024],
                    start=False, stop=(kt == 3),
                )

            def fc1_hook(mt):
                if mt >= 1:
                    fc1_chunk(mt - 1)

            conv3(3, h2pads[1], fc1_hook=fc1_hook)
            fc1_chunk(3)

            # relu in 256-col chunks alternating engines so the first
            # transpose starts ~350ns after psfA stops instead of 1.2us
            for q in range(4):
                ps_src = psfA if q < 2 else psfB
                off = 256 * (q % 2)
                if q % 2 == 0:
                    nc.scalar.activation(
                        out=z1[:, 256 * q : 256 * q + 256],
                        in_=ps_src[:, off : off + 256], func=RELU,
                    )
                else:
                    nc.vector.tensor_scalar(
                        out=z1[:, 256 * q : 256 * q + 256],
                        in0=ps_src[:, off : off + 256],
                        scalar1=0.0, scalar2=0.0, op0=ADD, op1=MAX,
                    )

            # z1 [8, 1024] -> z1T [128, 8, 8] via PE transpose (identity rhs),
            # drained by relu (idempotent) alternating engines; FC2 accumulates
            # over the 8 column chunks.
            psf2 = ps1_p.tile([128, 8], FP, name="ps_c1")
            nc.tensor.matmul(
                out=psf2[0:10, :], lhsT=fb1row[0:1, 1024:1034], rhs=ones[0:1, :],
                start=True, stop=False,
            )
            for c in range(8):
                zps = (ps23_p if c % 2 == 0 else ps3_p).tile(
                    [128, 8], FP, name="ps_c2" if c % 2 == 0 else "ps_c3"
                )
                nc.tensor.matmul(
                    out=zps[:, :], lhsT=z1[:, 128 * c : 128 * c + 128],
                    rhs=ident[0:8, 0:8], start=True, stop=True,
                )
                if c % 2 == 0:
                    nc.scalar.activation(out=z1T[:, c, :], in_=zps[:, :], func=RELU)
                else:
                    nc.vector.tensor_scalar(
                        out=z1T[:, c, :], in0=zps[:, :], scalar1=0.0, scalar2=0.0,
                        op0=ADD, op1=MAX,
                    )
                nc.tensor.matmul(
                    out=psf2[0:10, :],
                    lhsT=fw2t[:, c, :],
                    rhs=z1T[:, c, :],
                    start=False,
                    stop=(c == 7),
                )
            nc.scalar.activation(out=y_sb[:, :], in_=psf2[0:10, :], func=mybir.ActivationFunctionType.Copy)
            nc.sync.dma_start(out=outT_d[:, :], in_=y_sb[:, :])

    nc.compile()
    return nc


def _get_nc():
    if "nc" not in _cache:
        _cache["nc"] = _build()
    return _cache["nc"]


def kernel(**inputs):
    from concourse import bass_utils

    nc = _get_nc()
    xpad, weights = _prep(inputs)
    in_maps = [
        dict(weights, xpad=np.ascontiguousarray(xpad[8 * c : 8 * c + 8]))
        for c in range(8)
    ]
    res = bass_utils.run_bass_kernel_spmd(
        nc, in_maps, core_ids=list(range(8)), trace=TRACE
    )
    LAST["exec_time_ns"] = getattr(res, "exec_time_ns", None)
    LAST["profile_json"] = getattr(res, "profile_json", None)
    LAST["instructions_and_trace"] = getattr(res, "instructions_and_trace", None)
    out = np.concatenate([r["outT"].T for r in res.results], axis=0)
    return np.ascontiguousarray(out.astype(np.float32))


# revision 44
# speedup vs baseline: 1.0187x; 1.0031x over previous
import sys

import numpy as np
from ml_dtypes import bfloat16

sys.path.insert(0, "/opt/trn_rl_repo")

TRACE = False
LAST = {}
_cache = {}

SPARSITY = 0.5

# even columns first, then odd: makes the stride-2 convs read contiguously
_XPERM = np.r_[1:64:2, 0:64:2]


def _tf32(a):
    b = np.ascontiguousarray(np.asarray(a, np.float32))
    u = b.view(np.uint32).copy()
    u += np.uint32(0x0FFF) + ((u >> np.uint32(13)) & np.uint32(1))
    u &= np.uint32(0xFFFFE000)
    return u.view(np.float32)


def _masked(w, s):
    sa = np.abs(np.asarray(s, np.float32)).ravel()
    j = int((1.0 - SPARSITY) * sa.size)
    thr = np.partition(sa, j)[j]
    m = (np.abs(np.asarray(s, np.float32)) >= thr).astype(np.float32)
    return (np.asarray(w, np.float32) * m).astype(np.float32)


def _prep(inputs):
    w1m = _masked(inputs["w1"], inputs["s1"])  # [128,3,3,3]
    w2m = _masked(inputs["w2"], inputs["s2"])  # [256,128,3,3]
    w3m = _masked(inputs["w3"], inputs["s3"])  # [512,256,3,3]
    fw1m = _masked(inputs["fw1"], inputs["fs1"])  # [1024,512]
    fw2m = _masked(inputs["fw2"], inputs["fs2"])  # [10,1024]

    c = np.ascontiguousarray
    # conv1 as single K=27 matmul, K padded to 32 and replicated 4x across
    # partition groups so rhs tiles at base partitions 0/32/64/96 line up
    w1t = np.zeros((64, 128), np.float32)
    w1t[:27] = w1m.transpose(1, 2, 3, 0).reshape(27, 128)
    w1t[32:59] = w1t[:27]
    w2t = c(w2m.transpose(1, 2, 3, 0).reshape(128, 9 * 256))
    # mt-major: [k2, mt, kt*1152 + g*128 + o]
    w3t = c(
        w3m.reshape(4, 128, 2, 128, 3, 3)
        .transpose(3, 0, 2, 4, 5, 1)
        .reshape(128, 4, 2304)
    )
    # global-avg-pool 1/256 folded into fw1
    fw1t = c((fw1m.T.reshape(4, 128, 1024).transpose(1, 0, 2) / 256.0).astype(np.float32))
    fw2t = c(fw2m.T.reshape(8, 128, 10).transpose(1, 0, 2))

    weights = {
        "w1t": w1t.astype(bfloat16),
        "w2t": w2t.astype(bfloat16),
        "w3t": w3t.astype(bfloat16),
        "fw1t": fw1t.astype(bfloat16),
        "fw2t": fw2t.astype(bfloat16),
        "bias": np.concatenate(
            [
                np.asarray(inputs["b1"], np.float32).reshape(128, 1),
                np.asarray(inputs["b2"], np.float32).reshape(2, 128).T,
                np.asarray(inputs["b3"], np.float32).reshape(4, 128).T,
            ],
            axis=1,
        ),
        "fb1": np.concatenate(
            [np.asarray(inputs["fb1"], np.float32).reshape(1, 1024),
             np.asarray(inputs["fb2"], np.float32).reshape(1, 10),
             np.zeros((1, 6), np.float32)],
            axis=1,
        ).astype(bfloat16),
        "ident": np.eye(8, dtype=np.float32).astype(bfloat16),
    }
    xpad = np.zeros((64, 3, 66, 66), np.float32)
    xpad[:, :, 1:65, 1:65] = np.asarray(inputs["x"], np.float32)
    # im2col over (ch,ky,kx): x27[i, ch*9+ky*3+kx] = xpad[i, ch, ky:ky+64, kx:kx+64]
    x27 = np.empty((64, 27, 64, 64), np.float32)
    for ch in range(3):
        for ky in range(3):
            for kx in range(3):
                x27[:, ch * 9 + ky * 3 + kx] = xpad[:, ch, ky : ky + 64, kx : kx + 64]
    # pack for full-width DMA + phase-split columns:
    # xim32[i, half, 32*nt + k, r, px] = x27[i, k, 32*half + 8*nt + r, XPERM[px]]
    xr = x27[:, :, :, _XPERM].reshape(64, 27, 2, 4, 8, 64)  # [i,k,half,nt,r,px]
    # partition p = 32*a + k holds nt = 2*b + a of half h at free slot 2h+b:
    # matmul rhs bases stay at 0/32, one whole-image DMA per image.
    xim32 = np.zeros((64, 2, 32, 4, 8, 64), np.float32)  # [i,a,k,2h+b,r,px]
    for a in range(2):
        for h in range(2):
            for b in range(2):
                xim32[:, a, :27, 2 * h + b] = xr[:, :, h, 2 * b + a]
    xim32 = c(xim32.reshape(64, 64, 4, 8, 64).astype(bfloat16))
    return xim32, weights


def _build():
    import concourse.bacc as bacc
    import concourse.mybir as mybir
    import concourse.tile as tile

    FP = mybir.dt.float32
    FR = mybir.dt.float32r
    BF = mybir.dt.bfloat16
    RELU = mybir.ActivationFunctionType.Relu
    ADD = mybir.AluOpType.add
    MAX = mybir.AluOpType.max

    nc = bacc.Bacc("TRN2", target_bir_lowering=False, debug=False)

    xpad_d = nc.dram_tensor("xpad", [8, 64, 4, 8, 64], BF, kind="ExternalInput")
    w1t_d = nc.dram_tensor("w1t", [64, 128], BF, kind="ExternalInput")
    w2t_d = nc.dram_tensor("w2t", [128, 2304], BF, kind="ExternalInput")
    w3t_d = nc.dram_tensor("w3t", [128, 4, 2304], BF, kind="ExternalInput")
    fw1t_d = nc.dram_tensor("fw1t", [128, 4, 1024], BF, kind="ExternalInput")
    fw2t_d = nc.dram_tensor("fw2t", [128, 8, 10], BF, kind="ExternalInput")
    bias_d = nc.dram_tensor("bias", [128, 7], FP, kind="ExternalInput")
    fb1_d = nc.dram_tensor("fb1", [1, 1040], BF, kind="ExternalInput")
    ident_d = nc.dram_tensor("ident", [8, 8], BF, kind="ExternalInput")
    outT_d = nc.dram_tensor("outT", [10, 8], FP, kind="ExternalOutput")

    with tile.TileContext(nc) as tc:
        with tc.tile_pool(name="consts", bufs=1) as consts, \
             tc.tile_pool(name="xim_p", bufs=2) as xim_p, \
             tc.tile_pool(name="act_p", bufs=1) as act_p, \
             tc.tile_pool(name="h3_p", bufs=2) as h3_p, \
             tc.tile_pool(name="ps1_p", bufs=4, space="PSUM") as ps1_p, \
             tc.tile_pool(name="ps23_p", bufs=2, space="PSUM") as ps23_p, \
             tc.tile_pool(name="ps3_p", bufs=2, space="PSUM") as ps3_p:

            xims = {}

            def load_xim(img):
                t = xim_p.tile([64, 4, 8, 64], BF, name="ximg")
                nc.sync.dma_start(out=t[:, :, :, :], in_=xpad_d[img, :, :, :, :])
                xims[img] = t

            # DMA issue order = global transfer order: descriptor generation is
            # serialized (~0.7us each) on the Sync queue, so the tensors the
            # first matmuls need go first. Image 0 streams as two halves.
            xim00 = xim_p.tile([64, 2, 8, 64], BF, name="ximg")
            nc.sync.dma_start(out=xim00[:, :, :, :], in_=xpad_d[0, :, 0:2, :, :])
            w1t = consts.tile([64, 128], BF)
            nc.sync.dma_start(out=w1t[:, :], in_=w1t_d[:, :])
            xim01 = xim_p.tile([64, 2, 8, 64], BF, name="ximg")
            nc.sync.dma_start(out=xim01[:, :, :, :], in_=xpad_d[0, :, 2:4, :, :])
            xims[(0, 0)], xims[(0, 1)] = xim00, xim01
            bias = consts.tile([128, 7], FP)
            nc.sync.dma_start(out=bias[:, :], in_=bias_d[:, :])
            w2t = consts.tile([128, 2304], BF)
            nc.sync.dma_start(out=w2t[:, :], in_=w2t_d[:, :])
            load_xim(1)
            w3all = consts.tile([128, 4, 2304], BF)
            nc.sync.dma_start(out=w3all[:, :, :], in_=w3t_d[:, :, :])
            b1sb = bias[:, 0:1]
            b2sb = bias[:, 1:3]
            b3sb = bias[:, 3:7]

            # PE p-state warm-up: ramp runs on wall time since first dispatch,
            # so a burst of throwaway matmuls during the DMA wait gets the
            # engine to speed before conv1 starts.
            warm = consts.tile([32, 8, 64], BF)
            nc.vector.memset(warm[:, :, :], 0.0)
            ones = consts.tile([1, 8], BF)
            nc.vector.memset(ones[:, :], 1.0)
            for _ in range(4):
                wps = ps1_p.tile([128, 8, 64], FP, name="ps_c1")
                nc.tensor.matmul(
                    out=wps[0:64, :, :], lhsT=warm[:, 0, :], rhs=warm[:, :, :],
                    start=True, stop=True,
                )

            # h1 phase-split in one tile: cols 0:33 = even input cols
            # (0,2,..,64), cols 33:66 = odd (1,3,..,65). With the odds-first
            # pixel order from the host, a conv1 PSUM tile drains to
            # h1eo[:, rows, 1:65] in a single contiguous op.
            h1eo_a = act_p.tile([128, 66, 66], BF)
            h1eo_b = act_p.tile([128, 66, 66], BF)
            # h2 phase-split: h2e = cols 0,2,..,32 (17), h2o = cols 1,..,33 (17)
            h2e_a = act_p.tile([128, 2, 2, 34, 17], BF)
            h2o_a = act_p.tile([128, 2, 2, 34, 17], BF)
            h2e_b = act_p.tile([128, 2, 2, 34, 17], BF)
            h2o_b = act_p.tile([128, 2, 2, 34, 17], BF)
            hpool = act_p.tile([128, 4, 8], BF)
            z1 = act_p.tile([8, 1024], BF)
            z1T = act_p.tile([128, 8, 8], BF)
            y_sb = act_p.tile([10, 8], FP)

            # Border-only zeroing: interiors are fully overwritten every image,
            # borders stay zero for the kernel's lifetime.
            for h1eo in (h1eo_a, h1eo_b):
                nc.vector.memset(h1eo[:, 0, :], 0.0)
                nc.vector.memset(h1eo[:, 65, :], 0.0)
                nc.vector.memset(h1eo[:, 1:65, 0], 0.0)
                nc.vector.memset(h1eo[:, 1:65, 65], 0.0)
            for h2e, h2o in ((h2e_a, h2o_a), (h2e_b, h2o_b)):
                for m in range(2):
                    for i in range(2):
                        nc.vector.memset(h2e[:, m, i, 0, :], 0.0)
                        nc.vector.memset(h2e[:, m, i, 33, :], 0.0)
                        nc.vector.memset(h2e[:, m, i, 1:33, 0], 0.0)
                        nc.vector.memset(h2o[:, m, i, 0, :], 0.0)
                        nc.vector.memset(h2o[:, m, i, 33, :], 0.0)
                        nc.vector.memset(h2o[:, m, i, 1:33, 16], 0.0)

            h1pads = [h1eo_a, h1eo_b]
            h2pads = [(h2e_a, h2o_a), (h2e_b, h2o_b)]

            def drain(eng, out, ps, bias):
                if eng == 0:
                    nc.scalar.activation(out=out, in_=ps, func=RELU, bias=bias)
                else:
                    nc.vector.tensor_scalar(
                        out=out, in0=ps, scalar1=bias, scalar2=0.0,
                        op0=ADD, op1=MAX,
                    )

            def conv1_half(img, h1eo, half):
                if img == 0:
                    xim = xims.pop((0, half))
                    slot = 0
                else:
                    xim = xims.pop(img) if half == 1 else xims[img]
                    slot = 2 * half
                for nt in range(4):
                    ntg = 4 * half + nt
                    ps = ps1_p.tile([128, 8, 64], FP, name="ps_c1")
                    nc.tensor.matmul(
                        out=ps[:, :, :],
                        lhsT=w1t[32 * (nt % 2) : 32 * (nt % 2) + 32, :],
                        rhs=xim[32 * (nt % 2) : 32 * (nt % 2) + 32, slot + nt // 2, :, :],
                        start=True,
                        stop=True,
                    )
                    # odds-first pixel order: ps col j -> h1eo col j+1;
                    # split across both engines to halve the PSUM WAR latency
                    r0 = 1 + 8 * ntg
                    drain(ntg % 2, h1eo[:, r0 : r0 + 8, 1:33],
                          ps[:, :, 0:32], b1sb[:, 0:1])
                    drain(1 - ntg % 2, h1eo[:, r0 : r0 + 8, 33:65],
                          ps[:, :, 32:64], b1sb[:, 0:1])

            def conv2_half(img, h1eo, h2pair, islot, nh):
                h2e, h2o = h2pair
                # g-outer for rhs reuse; m=0 runs its last two passes early
                # so its drains overlap m=1's final matmuls and its PSUM slot
                # is free when the next half starts
                pss = [ps23_p.tile([128, 16, 32], FP, name="ps_c2") for _ in range(2)]
                # m1 trails m0 by 3 passes: the half opens m0-only (covering
                # the previous half's m1 drains) and closes m1-only (m0's
                # drains overlap), so neither PSUM slot is hot at a boundary
                order = [(0, 0), (1, 0), (2, 0)]
                for g in range(3, 9):
                    order += [(g, 0), (g - 3, 1)]
                order += [(6, 1), (7, 1), (8, 1)]

                def c2drain(m):
                    # out x' even -> h2 odd cols -> h2o[0:16]; odd -> h2e[1:17]
                    r0 = 1 + 16 * nh
                    drain(0 if m == 0 else 1, h2o[:, m, islot, r0 : r0 + 16, 0:16],
                          pss[m][:, :, 0:32:2], b2sb[:, m : m + 1])
                    drain(1 if m == 0 else 0, h2e[:, m, islot, r0 : r0 + 16, 1:17],
                          pss[m][:, :, 1:32:2], b2sb[:, m : m + 1])

                for g, m in order:
                    ky, kx = g // 3, g % 3
                    c0 = 0 if kx == 0 else 33 if kx == 1 else 1
                    r0 = 32 * nh + ky
                    nc.tensor.matmul(
                        out=pss[m][:, :, :],
                        lhsT=w2t[:, 256 * g + 128 * m : 256 * g + 128 * m + 128],
                        rhs=h1eo[:, r0 : r0 + 32 : 2, c0 : c0 + 32],
                        start=(g == 0),
                        stop=(g == 8),
                    )
                    if (g, m) == (8, 0):
                        c2drain(0)
                c2drain(1)

            def conv3(pair, h2pair, fc1_hook=None):
                h2e, h2o = h2pair
                for mt in range(4):
                    ps = ps3_p.tile([128, 2, 16, 16], FP, name="ps_c3")
                    n = 0
                    for kt in range(2):
                        for g in range(9):
                            ky, kx = g // 3, g % 3
                            hsrc, c0 = (
                                (h2e, 0) if kx == 0 else (h2o, 0) if kx == 1 else (h2e, 1)
                            )
                            nc.tensor.matmul(
                                out=ps[:, :, :, :],
                                lhsT=w3all[:, mt, 1152 * kt + 128 * g : 1152 * kt + 128 * g + 128],
                                rhs=hsrc[:, kt, :, ky : ky + 32 : 2, c0 : c0 + 16],
                                start=(n == 0),
                                stop=(n == 17),
                            )
                            n += 1
                    h3 = h3_p.tile([128, 2, 16, 16], FP, name="h3scr")
                    # accumulation runs in fp32 internally; only the final
                    # write is f32r-rounded (it feeds a tf32 matmul anyway)
                    with nc.allow_low_precision(reason="pool feeds f32r matmul"):
                        nc.scalar.activation(
                            out=h3[:, 0, :, :],
                            in_=ps[:, 0, :, :],
                            func=RELU,
                            bias=b3sb[:, mt : mt + 1],
                            accum_out=hpool[:, mt, 2 * pair : 2 * pair + 1],
                        )
                        nc.vector.tensor_scalar(
                            out=h3[:, 1, :, :], in0=ps[:, 1, :, :],
                            scalar1=b3sb[:, mt : mt + 1], scalar2=0.0,
                            op0=ADD, op1=MAX,
                        )
                        nc.vector.tensor_reduce(
                            out=hpool[:, mt, 2 * pair + 1 : 2 * pair + 2],
                            in_=h3[:, 1, :, :],
                            axis=mybir.AxisListType.XY,
                            op=ADD,
                        )
                    if fc1_hook is not None:
                        fc1_hook(mt)

            # conv3(pair p) is sandwiched between conv1 and conv2 of image
            # 2p+2: conv1's matmuls cover the latency of the last conv2
            # drains conv3 depends on, and conv3's long stretch covers
            # conv1's drains that conv2 depends on.
            fw1t = fw2t = fb1row = ident = None
            for img in range(8):
                pair, i = divmod(img, 2)
                h1pair = h1pads[img % 2]
                h2pair = h2pads[pair % 2]
                for half in range(2):
                    conv1_half(img, h1pair, half)
                    if half == 0 and img + 2 < 8:
                        load_xim(img + 2)
                if i == 0 and pair >= 1:
                    conv3(pair - 1, h2pads[(pair - 1) % 2])
                if img == 1:
                    fw1t = consts.tile([128, 4, 1024], BF)
                    nc.sync.dma_start(out=fw1t[:, :, :], in_=fw1t_d[:, :, :])
                    fw2t = consts.tile([128, 8, 10], BF)
                    nc.sync.dma_start(out=fw2t[:, :, :], in_=fw2t_d[:, :, :])
                    fb1row = consts.tile([1, 1040], BF)
                    nc.sync.dma_start(out=fb1row[:, :], in_=fb1_d[:, :])
                    ident = consts.tile([8, 8], BF)
                    nc.sync.dma_start(out=ident[:, :], in_=ident_d[:, :])
                for half in range(2):
                    conv2_half(img, h1pair, h2pair, i, half)

            # FC1 with batch on partitions: psf[b, j] = fb1[j] + sum_kt
            # hpool[:, kt, b]^T @ fw1t[:, kt, j]. One 512-wide matmul per
            # (kt, half): 4 LDWEIGHTS of hpool instead of 32 of fw1t, so the
            # PE isn't weight-load-bound. Bias lands first via a K=1 matmul
            # (all-ones lhsT). Chunk kt is issued one mt-group late inside
            # conv3(pair 3) so its hpool dependency is long satisfied and
            # only kt=3 remains on the serial tail.
            psfA = ps1_p.tile([8, 512], FP, name="ps_c1")
            psfB = ps1_p.tile([8, 512], FP, name="ps_c1")
            nc.tensor.matmul(
                out=psfA[:, :], lhsT=ones[0:1, :], rhs=fb1row[0:1, 0:512],
                start=True, stop=False,
            )
            nc.tensor.matmul(
                out=psfB[:, :], lhsT=ones[0:1, :], rhs=fb1row[0:1, 512:1024],
                start=True, stop=False,
            )

            def fc1_chunk(kt):
                nc.tensor.matmul(
                    out=psfA[:, :], lhsT=hpool[:, kt, :], rhs=fw1t[:, kt, 0:512],
                    start=False, stop=(kt == 3),
                )
                nc.tensor.matmul(
                    out=psfB[:, :], lhsT=hpool[:, kt, :], rhs=fw1t[:, kt, 512:1024],
                    start=False, stop=(kt == 3),
                )

            def fc1_hook(mt):
                if mt >= 1:
                    fc1_chunk(mt - 1)

            conv3(3, h2pads[1], fc1_hook=fc1_hook)
            fc1_chunk(3)

            # relu in 256-col chunks alternating engines so the first
            # transpose starts ~350ns after psfA stops instead of 1.2us
            for q in range(4):
                ps_src = psfA if q < 2 else psfB
                off = 256 * (q % 2)
                if q % 2 == 0:
                    nc.scalar.activation(
                        out=z1[:, 256 * q : 256 * q + 256],
                        in_=ps_src[:, off : off + 256], func=RELU,
                    )
                else:
                    nc.vector.tensor_scalar(
                        out=z1[:, 256 * q : 256 * q + 256],
                        in0=ps_src[:, off : off + 256],
                        scalar1=0.0, scalar2=0.0, op0=ADD, op1=MAX,
                    )

            # z1 [8, 1024] -> z1T [128, 8, 8] via PE transpose (identity rhs),
            # drained by relu (idempotent) alternating engines; FC2 accumulates
            # over the 8 column chunks.
            psf2 = ps1_p.tile([128, 8], FP, name="ps_c1")
            nc.tensor.matmul(
                out=psf2[0:10, :], lhsT=fb1row[0:1, 1024:1034], rhs=ones[0:1, :],
                start=True, stop=False,
            )
            for c in range(8):
                zps = (ps23_p if c % 2 == 0 else ps3_p).tile(
                    [128, 8], FP, name="ps_c2" if c % 2 == 0 else "ps_c3"
                )
                nc.tensor.matmul(
                    out=zps[:, :], lhsT=z1[:, 128 * c : 128 * c + 128],
                    rhs=ident[0:8, 0:8], start=True, stop=True,
                )
                if c % 2 == 0:
                    nc.scalar.activation(out=z1T[:, c, :], in_=zps[:, :], func=RELU)
                else:
                    nc.vector.tensor_scalar(
                        out=z1T[:, c, :], in0=zps[:, :], scalar1=0.0, scalar2=0.0,
                        op0=ADD, op1=MAX,
                    )
                nc.tensor.matmul(
                    out=psf2[0:10, :],
                    lhsT=fw2t[:, c, :],
                    rhs=z1T[:, c, :],
                    start=False,
                    stop=(c == 7),
                )
            nc.scalar.activation(out=y_sb[:, :], in_=psf2[0:10, :], func=mybir.ActivationFunctionType.Copy)
            nc.sync.dma_start(out=outT_d[:, :], in_=y_sb[:, :])

    nc.compile()
    return nc


def _get_nc():
    if "nc" not in _cache:
        _cache["nc"] = _build()
    return _cache["nc"]


def kernel(**inputs):
    from concourse import bass_utils

    nc = _get_nc()
    xpad, weights = _prep(inputs)
    in_maps = [
        dict(weights, xpad=np.ascontiguousarray(xpad[8 * c : 8 * c + 8]))
        for c in range(8)
    ]
    res = bass_utils.run_bass_kernel_spmd(
        nc, in_maps, core_ids=list(range(8)), trace=TRACE
    )
    LAST["exec_time_ns"] = getattr(res, "exec_time_ns", None)
    LAST["profile_json"] = getattr(res, "profile_json", None)
    LAST["instructions_and_trace"] = getattr(res, "instructions_and_trace", None)
    out = np.concatenate([r["outT"].T for r in res.results], axis=0)
    return np.ascontiguousarray(out.astype(np.float32))


# revision 45
# speedup vs baseline: 1.0191x; 1.0005x over previous
import sys

import numpy as np
from ml_dtypes import bfloat16

sys.path.insert(0, "/opt/trn_rl_repo")

TRACE = False
LAST = {}
_cache = {}

SPARSITY = 0.5

# even columns first, then odd: makes the stride-2 convs read contiguously
_XPERM = np.r_[1:64:2, 0:64:2]


def _tf32(a):
    b = np.ascontiguousarray(np.asarray(a, np.float32))
    u = b.view(np.uint32).copy()
    u += np.uint32(0x0FFF) + ((u >> np.uint32(13)) & np.uint32(1))
    u &= np.uint32(0xFFFFE000)
    return u.view(np.float32)


def _masked(w, s):
    sa = np.abs(np.asarray(s, np.float32)).ravel()
    j = int((1.0 - SPARSITY) * sa.size)
    thr = np.partition(sa, j)[j]
    m = (np.abs(np.asarray(s, np.float32)) >= thr).astype(np.float32)
    return (np.asarray(w, np.float32) * m).astype(np.float32)


def _prep(inputs):
    w1m = _masked(inputs["w1"], inputs["s1"])  # [128,3,3,3]
    w2m = _masked(inputs["w2"], inputs["s2"])  # [256,128,3,3]
    w3m = _masked(inputs["w3"], inputs["s3"])  # [512,256,3,3]
    fw1m = _masked(inputs["fw1"], inputs["fs1"])  # [1024,512]
    fw2m = _masked(inputs["fw2"], inputs["fs2"])  # [10,1024]

    c = np.ascontiguousarray
    # conv1 as single K=27 matmul, K padded to 32 and replicated 4x across
    # partition groups so rhs tiles at base partitions 0/32/64/96 line up
    w1t = np.zeros((64, 128), np.float32)
    w1t[:27] = w1m.transpose(1, 2, 3, 0).reshape(27, 128)
    w1t[32:59] = w1t[:27]
    w2t = c(w2m.transpose(1, 2, 3, 0).reshape(128, 9 * 256))
    # mt-major: [k2, mt, kt*1152 + g*128 + o]
    w3t = c(
        w3m.reshape(4, 128, 2, 128, 3, 3)
        .transpose(3, 0, 2, 4, 5, 1)
        .reshape(128, 4, 2304)
    )
    # global-avg-pool 1/256 folded into fw1
    fw1t = c((fw1m.T.reshape(4, 128, 1024).transpose(1, 0, 2) / 256.0).astype(np.float32))
    fw2t = c(fw2m.T.reshape(8, 128, 10).transpose(1, 0, 2))

    weights = {
        "w1t": w1t.astype(bfloat16),
        "w2t": w2t.astype(bfloat16),
        "w3t": w3t.astype(bfloat16),
        "fwall": None,  # filled below
        "bias": np.concatenate(
            [
                np.asarray(inputs["b1"], np.float32).reshape(128, 1),
                np.asarray(inputs["b2"], np.float32).reshape(2, 128).T,
                np.asarray(inputs["b3"], np.float32).reshape(4, 128).T,
            ],
            axis=1,
        ),
    }
    fwall = np.zeros((128, 5224), np.float32)
    fwall[:, 0:4096] = fw1t.reshape(128, 4096)
    fwall[:, 4096:4176] = fw2t.reshape(128, 80)
    fwall[0:8, 4176:4184] = np.eye(8, dtype=np.float32)
    fwall[0:1, 4184:5208] = np.asarray(inputs["fb1"], np.float32).reshape(1, 1024)
    fwall[0:1, 5208:5218] = np.asarray(inputs["fb2"], np.float32).reshape(1, 10)
    weights["fwall"] = fwall.astype(bfloat16)
    xpad = np.zeros((64, 3, 66, 66), np.float32)
    xpad[:, :, 1:65, 1:65] = np.asarray(inputs["x"], np.float32)
    # im2col over (ch,ky,kx): x27[i, ch*9+ky*3+kx] = xpad[i, ch, ky:ky+64, kx:kx+64]
    x27 = np.empty((64, 27, 64, 64), np.float32)
    for ch in range(3):
        for ky in range(3):
            for kx in range(3):
                x27[:, ch * 9 + ky * 3 + kx] = xpad[:, ch, ky : ky + 64, kx : kx + 64]
    # pack for full-width DMA + phase-split columns:
    # xim32[i, half, 32*nt + k, r, px] = x27[i, k, 32*half + 8*nt + r, XPERM[px]]
    xr = x27[:, :, :, _XPERM].reshape(64, 27, 2, 4, 8, 64)  # [i,k,half,nt,r,px]
    # partition p = 32*a + k holds nt = 2*b + a of half h at free slot 2h+b:
    # matmul rhs bases stay at 0/32, one whole-image DMA per image.
    xim32 = np.zeros((64, 2, 32, 4, 8, 64), np.float32)  # [i,a,k,2h+b,r,px]
    for a in range(2):
        for h in range(2):
            for b in range(2):
                xim32[:, a, :27, 2 * h + b] = xr[:, :, h, 2 * b + a]
    xim32 = c(xim32.reshape(64, 64, 4, 8, 64).astype(bfloat16))
    return xim32, weights


def _build():
    import concourse.bacc as bacc
    import concourse.mybir as mybir
    import concourse.tile as tile

    FP = mybir.dt.float32
    FR = mybir.dt.float32r
    BF = mybir.dt.bfloat16
    RELU = mybir.ActivationFunctionType.Relu
    ADD = mybir.AluOpType.add
    MAX = mybir.AluOpType.max

    nc = bacc.Bacc("TRN2", target_bir_lowering=False, debug=False)

    xpad_d = nc.dram_tensor("xpad", [8, 64, 4, 8, 64], BF, kind="ExternalInput")
    w1t_d = nc.dram_tensor("w1t", [64, 128], BF, kind="ExternalInput")
    w2t_d = nc.dram_tensor("w2t", [128, 2304], BF, kind="ExternalInput")
    w3t_d = nc.dram_tensor("w3t", [128, 4, 2304], BF, kind="ExternalInput")
    fwall_d = nc.dram_tensor("fwall", [128, 5224], BF, kind="ExternalInput")
    bias_d = nc.dram_tensor("bias", [128, 7], FP, kind="ExternalInput")
    outT_d = nc.dram_tensor("outT", [10, 8], FP, kind="ExternalOutput")

    with tile.TileContext(nc) as tc:
        with tc.tile_pool(name="consts", bufs=1) as consts, \
             tc.tile_pool(name="xim_p", bufs=2) as xim_p, \
             tc.tile_pool(name="act_p", bufs=1) as act_p, \
             tc.tile_pool(name="h3_p", bufs=2) as h3_p, \
             tc.tile_pool(name="ps1_p", bufs=4, space="PSUM") as ps1_p, \
             tc.tile_pool(name="ps23_p", bufs=2, space="PSUM") as ps23_p, \
             tc.tile_pool(name="ps3_p", bufs=2, space="PSUM") as ps3_p:

            xims = {}

            def load_xim(img):
                t = xim_p.tile([64, 4, 8, 64], BF, name="ximg")
                nc.sync.dma_start(out=t[:, :, :, :], in_=xpad_d[img, :, :, :, :])
                xims[img] = t

            # DMA issue order = global transfer order: descriptor generation is
            # serialized (~0.7us each) on the Sync queue, so the tensors the
            # first matmuls need go first. Image 0 streams as two halves.
            xim00 = xim_p.tile([64, 2, 8, 64], BF, name="ximg")
            nc.sync.dma_start(out=xim00[:, :, :, :], in_=xpad_d[0, :, 0:2, :, :])
            w1t = consts.tile([64, 128], BF)
            nc.sync.dma_start(out=w1t[:, :], in_=w1t_d[:, :])
            xim01 = xim_p.tile([64, 2, 8, 64], BF, name="ximg")
            nc.sync.dma_start(out=xim01[:, :, :, :], in_=xpad_d[0, :, 2:4, :, :])
            xims[(0, 0)], xims[(0, 1)] = xim00, xim01
            bias = consts.tile([128, 7], FP)
            nc.sync.dma_start(out=bias[:, :], in_=bias_d[:, :])
            w2t = consts.tile([128, 2304], BF)
            nc.sync.dma_start(out=w2t[:, :], in_=w2t_d[:, :])
            load_xim(1)
            w3all = consts.tile([128, 4, 2304], BF)
            nc.sync.dma_start(out=w3all[:, :, :], in_=w3t_d[:, :, :])
            b1sb = bias[:, 0:1]
            b2sb = bias[:, 1:3]
            b3sb = bias[:, 3:7]

            # PE p-state warm-up: ramp runs on wall time since first dispatch,
            # so a burst of throwaway matmuls during the DMA wait gets the
            # engine to speed before conv1 starts.
            warm = consts.tile([32, 8, 64], BF)
            nc.vector.memset(warm[:, :, :], 0.0)
            ones = consts.tile([1, 8], BF)
            nc.vector.memset(ones[:, :], 1.0)
            for _ in range(4):
                wps = ps1_p.tile([128, 8, 64], FP, name="ps_c1")
                nc.tensor.matmul(
                    out=wps[0:64, :, :], lhsT=warm[:, 0, :], rhs=warm[:, :, :],
                    start=True, stop=True,
                )

            # h1 phase-split in one tile: cols 0:33 = even input cols
            # (0,2,..,64), cols 33:66 = odd (1,3,..,65). With the odds-first
            # pixel order from the host, a conv1 PSUM tile drains to
            # h1eo[:, rows, 1:65] in a single contiguous op.
            h1eo_a = act_p.tile([128, 66, 66], BF)
            h1eo_b = act_p.tile([128, 66, 66], BF)
            # h2 phase-split: h2e = cols 0,2,..,32 (17), h2o = cols 1,..,33 (17)
            h2e_a = act_p.tile([128, 2, 2, 34, 17], BF)
            h2o_a = act_p.tile([128, 2, 2, 34, 17], BF)
            h2e_b = act_p.tile([128, 2, 2, 34, 17], BF)
            h2o_b = act_p.tile([128, 2, 2, 34, 17], BF)
            hpool = act_p.tile([128, 4, 8], BF)
            z1 = act_p.tile([8, 1024], BF)
            z1T = act_p.tile([128, 8, 8], BF)
            y_sb = act_p.tile([10, 8], FP)

            # Border-only zeroing: interiors are fully overwritten every image,
            # borders stay zero for the kernel's lifetime.
            for h1eo in (h1eo_a, h1eo_b):
                nc.vector.memset(h1eo[:, 0, :], 0.0)
                nc.vector.memset(h1eo[:, 65, :], 0.0)
                nc.vector.memset(h1eo[:, 1:65, 0], 0.0)
                nc.vector.memset(h1eo[:, 1:65, 65], 0.0)
            for h2e, h2o in ((h2e_a, h2o_a), (h2e_b, h2o_b)):
                for m in range(2):
                    for i in range(2):
                        nc.vector.memset(h2e[:, m, i, 0, :], 0.0)
                        nc.vector.memset(h2e[:, m, i, 33, :], 0.0)
                        nc.vector.memset(h2e[:, m, i, 1:33, 0], 0.0)
                        nc.vector.memset(h2o[:, m, i, 0, :], 0.0)
                        nc.vector.memset(h2o[:, m, i, 33, :], 0.0)
                        nc.vector.memset(h2o[:, m, i, 1:33, 16], 0.0)

            h1pads = [h1eo_a, h1eo_b]
            h2pads = [(h2e_a, h2o_a), (h2e_b, h2o_b)]

            def drain(eng, out, ps, bias):
                if eng == 0:
                    nc.scalar.activation(out=out, in_=ps, func=RELU, bias=bias)
                else:
                    nc.vector.tensor_scalar(
                        out=out, in0=ps, scalar1=bias, scalar2=0.0,
                        op0=ADD, op1=MAX,
                    )

            def conv1_half(img, h1eo, half):
                if img == 0:
                    xim = xims.pop((0, half))
                    slot = 0
                else:
                    xim = xims.pop(img) if half == 1 else xims[img]
                    slot = 2 * half
                for nt in range(4):
                    ntg = 4 * half + nt
                    ps = ps1_p.tile([128, 8, 64], FP, name="ps_c1")
                    nc.tensor.matmul(
                        out=ps[:, :, :],
                        lhsT=w1t[32 * (nt % 2) : 32 * (nt % 2) + 32, :],
                        rhs=xim[32 * (nt % 2) : 32 * (nt % 2) + 32, slot + nt // 2, :, :],
                        start=True,
                        stop=True,
                    )
                    # odds-first pixel order: ps col j -> h1eo col j+1;
                    # split across both engines to halve the PSUM WAR latency
                    r0 = 1 + 8 * ntg
                    drain(ntg % 2, h1eo[:, r0 : r0 + 8, 1:33],
                          ps[:, :, 0:32], b1sb[:, 0:1])
                    drain(1 - ntg % 2, h1eo[:, r0 : r0 + 8, 33:65],
                          ps[:, :, 32:64], b1sb[:, 0:1])

            def conv2_half(img, h1eo, h2pair, islot, nh):
                h2e, h2o = h2pair
                # g-outer for rhs reuse; m=0 runs its last two passes early
                # so its drains overlap m=1's final matmuls and its PSUM slot
                # is free when the next half starts
                pss = [ps23_p.tile([128, 16, 32], FP, name="ps_c2") for _ in range(2)]
                # m1 trails m0 by 3 passes: the half opens m0-only (covering
                # the previous half's m1 drains) and closes m1-only (m0's
                # drains overlap), so neither PSUM slot is hot at a boundary
                order = [(0, 0), (1, 0), (2, 0)]
                for g in range(3, 9):
                    order += [(g, 0), (g - 3, 1)]
                order += [(6, 1), (7, 1), (8, 1)]

                def c2drain(m):
                    # out x' even -> h2 odd cols -> h2o[0:16]; odd -> h2e[1:17]
                    r0 = 1 + 16 * nh
                    drain(0 if m == 0 else 1, h2o[:, m, islot, r0 : r0 + 16, 0:16],
                          pss[m][:, :, 0:32:2], b2sb[:, m : m + 1])
                    drain(1 if m == 0 else 0, h2e[:, m, islot, r0 : r0 + 16, 1:17],
                          pss[m][:, :, 1:32:2], b2sb[:, m : m + 1])

                for g, m in order:
                    ky, kx = g // 3, g % 3
                    c0 = 0 if kx == 0 else 33 if kx == 1 else 1
                    r0 = 32 * nh + ky
                    nc.tensor.matmul(
                        out=pss[m][:, :, :],
                        lhsT=w2t[:, 256 * g + 128 * m : 256 * g + 128 * m + 128],
                        rhs=h1eo[:, r0 : r0 + 32 : 2, c0 : c0 + 32],
                        start=(g == 0),
                        stop=(g == 8),
                    )
                    if (g, m) == (8, 0):
                        c2drain(0)
                c2drain(1)

            def conv3(pair, h2pair, fc1_hook=None):
                h2e, h2o = h2pair
                for mt in range(4):
                    ps = ps3_p.tile([128, 2, 16, 16], FP, name="ps_c3")
                    n = 0
                    for kt in range(2):
                        for g in range(9):
                            ky, kx = g // 3, g % 3
                            hsrc, c0 = (
                                (h2e, 0) if kx == 0 else (h2o, 0) if kx == 1 else (h2e, 1)
                            )
                            nc.tensor.matmul(
                                out=ps[:, :, :, :],
                                lhsT=w3all[:, mt, 1152 * kt + 128 * g : 1152 * kt + 128 * g + 128],
                                rhs=hsrc[:, kt, :, ky : ky + 32 : 2, c0 : c0 + 16],
                                start=(n == 0),
                                stop=(n == 17),
                            )
                            n += 1
                    h3 = h3_p.tile([128, 2, 16, 16], FP, name="h3scr")
                    # accumulation runs in fp32 internally; only the final
                    # write is f32r-rounded (it feeds a tf32 matmul anyway)
                    with nc.allow_low_precision(reason="pool feeds f32r matmul"):
                        nc.scalar.activation(
                            out=h3[:, 0, :, :],
                            in_=ps[:, 0, :, :],
                            func=RELU,
                            bias=b3sb[:, mt : mt + 1],
                            accum_out=hpool[:, mt, 2 * pair : 2 * pair + 1],
                        )
                        nc.vector.tensor_scalar(
                            out=h3[:, 1, :, :], in0=ps[:, 1, :, :],
                            scalar1=b3sb[:, mt : mt + 1], scalar2=0.0,
                            op0=ADD, op1=MAX,
                        )
                        nc.vector.tensor_reduce(
                            out=hpool[:, mt, 2 * pair + 1 : 2 * pair + 2],
                            in_=h3[:, 1, :, :],
                            axis=mybir.AxisListType.XY,
                            op=ADD,
                        )
                    if fc1_hook is not None:
                        fc1_hook(mt)

            # conv3(pair p) is sandwiched between conv1 and conv2 of image
            # 2p+2: conv1's matmuls cover the latency of the last conv2
            # drains conv3 depends on, and conv3's long stretch covers
            # conv1's drains that conv2 depends on.
            fwall = None
            for img in range(8):
                pair, i = divmod(img, 2)
                h1pair = h1pads[img % 2]
                h2pair = h2pads[pair % 2]
                for half in range(2):
                    conv1_half(img, h1pair, half)
                    if half == 0 and img + 2 < 8:
                        load_xim(img + 2)
                if i == 0 and pair >= 1:
                    conv3(pair - 1, h2pads[(pair - 1) % 2])
                if img == 1:
                    fwall = consts.tile([128, 5224], BF)
                    nc.sync.dma_start(out=fwall[:, :], in_=fwall_d[:, :])
                for half in range(2):
                    conv2_half(img, h1pair, h2pair, i, half)

            # FC1 with batch on partitions: psf[b, j] = fb1[j] + sum_kt
            # hpool[:, kt, b]^T @ fw1t[:, kt, j]. One 512-wide matmul per
            # (kt, half): 4 LDWEIGHTS of hpool instead of 32 of fw1t, so the
            # PE isn't weight-load-bound. Bias lands first via a K=1 matmul
            # (all-ones lhsT). Chunk kt is issued one mt-group late inside
            # conv3(pair 3) so its hpool dependency is long satisfied and
            # only kt=3 remains on the serial tail.
            psfA = ps1_p.tile([8, 512], FP, name="ps_c1")
            psfB = ps1_p.tile([8, 512], FP, name="ps_c1")
            nc.tensor.matmul(
                out=psfA[:, :], lhsT=ones[0:1, :], rhs=fwall[0:1, 4184:4696],
                start=True, stop=False,
            )
            nc.tensor.matmul(
                out=psfB[:, :], lhsT=ones[0:1, :], rhs=fwall[0:1, 4696:5208],
                start=True, stop=False,
            )

            def fc1_chunk(kt):
                nc.tensor.matmul(
                    out=psfA[:, :], lhsT=hpool[:, kt, :],
                    rhs=fwall[:, 1024 * kt : 1024 * kt + 512],
                    start=False, stop=(kt == 3),
                )
                nc.tensor.matmul(
                    out=psfB[:, :], lhsT=hpool[:, kt, :],
                    rhs=fwall[:, 1024 * kt + 512 : 1024 * kt + 1024],
                    start=False, stop=(kt == 3),
                )

            def fc1_hook(mt):
                if mt >= 1:
                    fc1_chunk(mt - 1)

            conv3(3, h2pads[1], fc1_hook=fc1_hook)
            fc1_chunk(3)

            # relu in 256-col chunks alternating engines so the first
            # transpose starts ~350ns after psfA stops instead of 1.2us
            for q in range(4):
                ps_src = psfA if q < 2 else psfB
                off = 256 * (q % 2)
                if q % 2 == 0:
                    nc.scalar.activation(
                        out=z1[:, 256 * q : 256 * q + 256],
                        in_=ps_src[:, off : off + 256], func=RELU,
                    )
                else:
                    nc.vector.tensor_scalar(
                        out=z1[:, 256 * q : 256 * q + 256],
                        in0=ps_src[:, off : off + 256],
                        scalar1=0.0, scalar2=0.0, op0=ADD, op1=MAX,
                    )

            # z1 [8, 1024] -> z1T [128, 8, 8] via PE transpose (identity rhs),
            # drained by relu (idempotent) alternating engines; FC2 accumulates
            # over the 8 column chunks.
            psf2 = ps1_p.tile([128, 8], FP, name="ps_c1")
            nc.tensor.matmul(
                out=psf2[0:10, :], lhsT=fwall[0:1, 5208:5218], rhs=ones[0:1, :],
                start=True, stop=False,
            )
            for c in range(8):
                zps = (ps23_p if c % 2 == 0 else ps3_p).tile(
                    [128, 8], FP, name="ps_c2" if c % 2 == 0 else "ps_c3"
                )
                nc.tensor.matmul(
                    out=zps[:, :], lhsT=z1[:, 128 * c : 128 * c + 128],
                    rhs=fwall[0:8, 4176:4184], start=True, stop=True,
                )
                if c % 2 == 0:
                    nc.scalar.activation(out=z1T[:, c, :], in_=zps[:, :], func=RELU)
                else:
                    nc.vector.tensor_scalar(
                        out=z1T[:, c, :], in0=zps[:, :], scalar1=0.0, scalar2=0.0,
                        op0=ADD, op1=MAX,
                    )
                nc.tensor.matmul(
                    out=psf2[0:10, :],
                    lhsT=fwall[:, 4096 + 10 * c : 4096 + 10 * c + 10],
                    rhs=z1T[:, c, :],
                    start=False,
                    stop=(c == 7),
                )
            nc.scalar.activation(out=y_sb[:, :], in_=psf2[0:10, :], func=mybir.ActivationFunctionType.Copy)
            nc.sync.dma_start(out=outT_d[:, :], in_=y_sb[:, :])

    nc.compile()
    return nc


def _get_nc():
    if "nc" not in _cache:
        _cache["nc"] = _build()
    return _cache["nc"]


def kernel(**inputs):
    from concourse import bass_utils

    nc = _get_nc()
    xpad, weights = _prep(inputs)
    in_maps = [
        dict(weights, xpad=np.ascontiguousarray(xpad[8 * c : 8 * c + 8]))
        for c in range(8)
    ]
    res = bass_utils.run_bass_kernel_spmd(
        nc, in_maps, core_ids=list(range(8)), trace=TRACE
    )
    LAST["exec_time_ns"] = getattr(res, "exec_time_ns", None)
    LAST["profile_json"] = getattr(res, "profile_json", None)
    LAST["instructions_and_trace"] = getattr(res, "instructions_and_trace", None)
    out = np.concatenate([r["outT"].T for r in res.results], axis=0)
    return np.ascontiguousarray(out.astype(np.float32))


# revision 47
# speedup vs baseline: 1.0213x; 1.0022x over previous
import sys

import numpy as np
from ml_dtypes import bfloat16

sys.path.insert(0, "/opt/trn_rl_repo")

TRACE = False
LAST = {}
_cache = {}

SPARSITY = 0.5

# even columns first, then odd: makes the stride-2 convs read contiguously
_XPERM = np.r_[1:64:2, 0:64:2]


def _tf32(a):
    b = np.ascontiguousarray(np.asarray(a, np.float32))
    u = b.view(np.uint32).copy()
    u += np.uint32(0x0FFF) + ((u >> np.uint32(13)) & np.uint32(1))
    u &= np.uint32(0xFFFFE000)
    return u.view(np.float32)


def _masked(w, s):
    sa = np.abs(np.asarray(s, np.float32)).ravel()
    j = int((1.0 - SPARSITY) * sa.size)
    thr = np.partition(sa, j)[j]
    m = (np.abs(np.asarray(s, np.float32)) >= thr).astype(np.float32)
    return (np.asarray(w, np.float32) * m).astype(np.float32)


def _prep(inputs):
    w1m = _masked(inputs["w1"], inputs["s1"])  # [128,3,3,3]
    w2m = _masked(inputs["w2"], inputs["s2"])  # [256,128,3,3]
    w3m = _masked(inputs["w3"], inputs["s3"])  # [512,256,3,3]
    fw1m = _masked(inputs["fw1"], inputs["fs1"])  # [1024,512]
    fw2m = _masked(inputs["fw2"], inputs["fs2"])  # [10,1024]

    c = np.ascontiguousarray
    # conv1 as single K=27 matmul, K padded to 32 and replicated 4x across
    # partition groups so rhs tiles at base partitions 0/32/64/96 line up
    w1t = np.zeros((64, 128), np.float32)
    w1t[:27] = w1m.transpose(1, 2, 3, 0).reshape(27, 128)
    w1t[32:59] = w1t[:27]
    w2t = c(w2m.transpose(1, 2, 3, 0).reshape(128, 9 * 256))
    # mt-major: [k2, mt, kt*1152 + g*128 + o]
    w3t = c(
        w3m.reshape(4, 128, 2, 128, 3, 3)
        .transpose(3, 0, 2, 4, 5, 1)
        .reshape(128, 4, 2304)
    )
    # global-avg-pool 1/256 folded into fw1
    fw1t = c((fw1m.T.reshape(4, 128, 1024).transpose(1, 0, 2) / 256.0).astype(np.float32))
    fw2t = c(fw2m.T.reshape(8, 128, 10).transpose(1, 0, 2))

    weights = {
        "w1t": w1t.astype(bfloat16),
        "w2t": w2t.astype(bfloat16),
        "w3t": w3t.astype(bfloat16),
        "fwall": None,  # filled below
        "bias": np.concatenate(
            [
                np.asarray(inputs["b1"], np.float32).reshape(128, 1),
                np.asarray(inputs["b2"], np.float32).reshape(2, 128).T,
                np.asarray(inputs["b3"], np.float32).reshape(4, 128).T,
            ],
            axis=1,
        ),
    }
    fwall = np.zeros((128, 5224), np.float32)
    fwall[:, 0:4096] = fw1t.reshape(128, 4096)
    fwall[:, 4096:4176] = fw2t.reshape(128, 80)
    fwall[0:8, 4176:4184] = np.eye(8, dtype=np.float32)
    fwall[0:1, 4184:5208] = np.asarray(inputs["fb1"], np.float32).reshape(1, 1024)
    fwall[0:1, 5208:5218] = np.asarray(inputs["fb2"], np.float32).reshape(1, 10)
    weights["fwall"] = fwall.astype(bfloat16)
    xpad = np.zeros((64, 3, 66, 66), np.float32)
    xpad[:, :, 1:65, 1:65] = np.asarray(inputs["x"], np.float32)
    # im2col over (ch,ky,kx): x27[i, ch*9+ky*3+kx] = xpad[i, ch, ky:ky+64, kx:kx+64]
    x27 = np.empty((64, 27, 64, 64), np.float32)
    for ch in range(3):
        for ky in range(3):
            for kx in range(3):
                x27[:, ch * 9 + ky * 3 + kx] = xpad[:, ch, ky : ky + 64, kx : kx + 64]
    # pack for full-width DMA + phase-split columns:
    # xim32[i, half, 32*nt + k, r, px] = x27[i, k, 32*half + 8*nt + r, XPERM[px]]
    xr = x27[:, :, :, _XPERM].reshape(64, 27, 2, 4, 8, 64)  # [i,k,half,nt,r,px]
    # partition p = 32*a + k holds nt = 2*b + a of half h at free slot 2h+b:
    # matmul rhs bases stay at 0/32, one whole-image DMA per image.
    xim32 = np.zeros((64, 2, 32, 4, 8, 64), np.float32)  # [i,a,k,2h+b,r,px]
    for a in range(2):
        for h in range(2):
            for b in range(2):
                xim32[:, a, :27, 2 * h + b] = xr[:, :, h, 2 * b + a]
    xim32 = c(xim32.reshape(64, 64, 4, 8, 64).astype(bfloat16))
    return xim32, weights


def _build():
    import concourse.bacc as bacc
    import concourse.mybir as mybir
    import concourse.tile as tile

    FP = mybir.dt.float32
    FR = mybir.dt.float32r
    BF = mybir.dt.bfloat16
    RELU = mybir.ActivationFunctionType.Relu
    ADD = mybir.AluOpType.add
    MAX = mybir.AluOpType.max

    nc = bacc.Bacc("TRN2", target_bir_lowering=False, debug=False)

    xpad_d = nc.dram_tensor("xpad", [8, 64, 4, 8, 64], BF, kind="ExternalInput")
    w1t_d = nc.dram_tensor("w1t", [64, 128], BF, kind="ExternalInput")
    w2t_d = nc.dram_tensor("w2t", [128, 2304], BF, kind="ExternalInput")
    w3t_d = nc.dram_tensor("w3t", [128, 4, 2304], BF, kind="ExternalInput")
    fwall_d = nc.dram_tensor("fwall", [128, 5224], BF, kind="ExternalInput")
    bias_d = nc.dram_tensor("bias", [128, 7], FP, kind="ExternalInput")
    outT_d = nc.dram_tensor("outT", [10, 8], FP, kind="ExternalOutput")

    with tile.TileContext(nc) as tc:
        with tc.tile_pool(name="consts", bufs=1) as consts, \
             tc.tile_pool(name="xim_p", bufs=2) as xim_p, \
             tc.tile_pool(name="act_p", bufs=1) as act_p, \
             tc.tile_pool(name="h3_p", bufs=2) as h3_p, \
             tc.tile_pool(name="ps1_p", bufs=4, space="PSUM") as ps1_p, \
             tc.tile_pool(name="ps23_p", bufs=2, space="PSUM") as ps23_p, \
             tc.tile_pool(name="ps3_p", bufs=2, space="PSUM") as ps3_p:

            xims = {}

            def load_xim(img):
                t = xim_p.tile([64, 4, 8, 64], BF, name="ximg")
                nc.sync.dma_start(out=t[:, :, :, :], in_=xpad_d[img, :, :, :, :])
                xims[img] = t

            # DMA issue order = global transfer order: descriptor generation is
            # serialized (~0.7us each) on the Sync queue, so the tensors the
            # first matmuls need go first. Image 0 streams as two halves.
            xim00 = xim_p.tile([64, 2, 8, 64], BF, name="ximg")
            nc.sync.dma_start(out=xim00[:, :, :, :], in_=xpad_d[0, :, 0:2, :, :])
            w1t = consts.tile([64, 128], BF)
            nc.sync.dma_start(out=w1t[:, :], in_=w1t_d[:, :])
            xim01 = xim_p.tile([64, 2, 8, 64], BF, name="ximg")
            nc.sync.dma_start(out=xim01[:, :, :, :], in_=xpad_d[0, :, 2:4, :, :])
            xims[(0, 0)], xims[(0, 1)] = xim00, xim01
            bias = consts.tile([128, 7], FP)
            nc.sync.dma_start(out=bias[:, :], in_=bias_d[:, :])
            w2t = consts.tile([128, 2304], BF)
            nc.sync.dma_start(out=w2t[:, :], in_=w2t_d[:, :])
            load_xim(1)
            w3all = consts.tile([128, 4, 2304], BF)
            nc.sync.dma_start(out=w3all[:, :, :], in_=w3t_d[:, :, :])
            b1sb = bias[:, 0:1]
            b2sb = bias[:, 1:3]
            b3sb = bias[:, 3:7]

            # PE p-state warm-up: ramp runs on wall time since first dispatch,
            # so a burst of throwaway matmuls during the DMA wait gets the
            # engine to speed before conv1 starts.
            warm = consts.tile([32, 8, 64], BF)
            nc.vector.memset(warm[:, :, :], 0.0)
            ones = consts.tile([1, 8], BF)
            nc.vector.memset(ones[:, :], 1.0)
            for _ in range(4):
                wps = ps1_p.tile([128, 8, 64], FP, name="ps_c1")
                nc.tensor.matmul(
                    out=wps[0:64, :, :], lhsT=warm[:, 0, :], rhs=warm[:, :, :],
                    start=True, stop=True,
                )

            # h1 phase-split in one tile: cols 0:33 = even input cols
            # (0,2,..,64), cols 33:66 = odd (1,3,..,65). With the odds-first
            # pixel order from the host, a conv1 PSUM tile drains to
            # h1eo[:, rows, 1:65] in a single contiguous op.
            h1eo_a = act_p.tile([128, 66, 66], BF)
            h1eo_b = act_p.tile([128, 66, 66], BF)
            # h2 phase-split: h2e = cols 0,2,..,32 (17), h2o = cols 1,..,33 (17)
            h2e_a = act_p.tile([128, 2, 2, 34, 17], BF)
            h2o_a = act_p.tile([128, 2, 2, 34, 17], BF)
            h2e_b = act_p.tile([128, 2, 2, 34, 17], BF)
            h2o_b = act_p.tile([128, 2, 2, 34, 17], BF)
            hpool = act_p.tile([128, 4, 8], BF)
            z1 = act_p.tile([8, 1024], BF)
            z1T = act_p.tile([128, 8, 8], BF)
            y_sb = act_p.tile([10, 8], FP)

            # Border-only zeroing: interiors are fully overwritten every image,
            # borders stay zero for the kernel's lifetime.
            for h1eo in (h1eo_a, h1eo_b):
                nc.vector.memset(h1eo[:, 0, :], 0.0)
                nc.vector.memset(h1eo[:, 65, :], 0.0)
                nc.vector.memset(h1eo[:, 1:65, 0], 0.0)
                nc.vector.memset(h1eo[:, 1:65, 65], 0.0)
            for h2e, h2o in ((h2e_a, h2o_a), (h2e_b, h2o_b)):
                for m in range(2):
                    for i in range(2):
                        nc.vector.memset(h2e[:, m, i, 0, :], 0.0)
                        nc.vector.memset(h2e[:, m, i, 33, :], 0.0)
                        nc.vector.memset(h2e[:, m, i, 1:33, 0], 0.0)
                        nc.vector.memset(h2o[:, m, i, 0, :], 0.0)
                        nc.vector.memset(h2o[:, m, i, 33, :], 0.0)
                        nc.vector.memset(h2o[:, m, i, 1:33, 16], 0.0)

            h1pads = [h1eo_a, h1eo_b]
            h2pads = [(h2e_a, h2o_a), (h2e_b, h2o_b)]

            def drain(eng, out, ps, bias):
                if eng == 0:
                    nc.scalar.activation(out=out, in_=ps, func=RELU, bias=bias)
                else:
                    nc.vector.tensor_scalar(
                        out=out, in0=ps, scalar1=bias, scalar2=0.0,
                        op0=ADD, op1=MAX,
                    )

            def conv1_half(img, h1eo, half):
                if img == 0:
                    xim = xims.pop((0, half))
                    slot = 0
                else:
                    xim = xims.pop(img) if half == 1 else xims[img]
                    slot = 2 * half
                for nt in range(4):
                    ntg = 4 * half + nt
                    ps = ps1_p.tile([128, 8, 64], FP, name="ps_c1")
                    nc.tensor.matmul(
                        out=ps[:, :, :],
                        lhsT=w1t[32 * (nt % 2) : 32 * (nt % 2) + 32, :],
                        rhs=xim[32 * (nt % 2) : 32 * (nt % 2) + 32, slot + nt // 2, :, :],
                        start=True,
                        stop=True,
                    )
                    # odds-first pixel order: ps col j -> h1eo col j+1;
                    # split across both engines to halve the PSUM WAR latency
                    r0 = 1 + 8 * ntg
                    drain(ntg % 2, h1eo[:, r0 : r0 + 8, 1:33],
                          ps[:, :, 0:32], b1sb[:, 0:1])
                    drain(1 - ntg % 2, h1eo[:, r0 : r0 + 8, 33:65],
                          ps[:, :, 32:64], b1sb[:, 0:1])

            def conv2_half(img, h1eo, h2pair, islot, nh):
                h2e, h2o = h2pair
                # g-outer for rhs reuse; m=0 runs its last two passes early
                # so its drains overlap m=1's final matmuls and its PSUM slot
                # is free when the next half starts
                pss = [ps23_p.tile([128, 16, 32], FP, name="ps_c2") for _ in range(2)]
                # m1 trails m0 by 3 passes: the half opens m0-only (covering
                # the previous half's m1 drains) and closes m1-only (m0's
                # drains overlap), so neither PSUM slot is hot at a boundary
                order = [(0, 0), (1, 0), (2, 0)]
                for g in range(3, 9):
                    order += [(g, 0), (g - 3, 1)]
                order += [(6, 1), (7, 1), (8, 1)]

                def c2drain(m):
                    # out x' even -> h2 odd cols -> h2o[0:16]; odd -> h2e[1:17]
                    r0 = 1 + 16 * nh
                    drain(0 if m == 0 else 1, h2o[:, m, islot, r0 : r0 + 16, 0:16],
                          pss[m][:, :, 0:32:2], b2sb[:, m : m + 1])
                    drain(1 if m == 0 else 0, h2e[:, m, islot, r0 : r0 + 16, 1:17],
                          pss[m][:, :, 1:32:2], b2sb[:, m : m + 1])

                for g, m in order:
                    ky, kx = g // 3, g % 3
                    c0 = 0 if kx == 0 else 33 if kx == 1 else 1
                    r0 = 32 * nh + ky
                    nc.tensor.matmul(
                        out=pss[m][:, :, :],
                        lhsT=w2t[:, 256 * g + 128 * m : 256 * g + 128 * m + 128],
                        rhs=h1eo[:, r0 : r0 + 32 : 2, c0 : c0 + 32],
                        start=(g == 0),
                        stop=(g == 8),
                    )
                    if (g, m) == (8, 0):
                        c2drain(0)
                c2drain(1)

            def conv3(pair, h2pair, fc1_hook=None):
                h2e, h2o = h2pair
                for mt in range(4):
                    ps = ps3_p.tile([128, 2, 16, 16], FP, name="ps_c3")
                    n = 0
                    for kt in range(2):
                        for g in range(9):
                            ky, kx = g // 3, g % 3
                            hsrc, c0 = (
                                (h2e, 0) if kx == 0 else (h2o, 0) if kx == 1 else (h2e, 1)
                            )
                            nc.tensor.matmul(
                                out=ps[:, :, :, :],
                                lhsT=w3all[:, mt, 1152 * kt + 128 * g : 1152 * kt + 128 * g + 128],
                                rhs=hsrc[:, kt, :, ky : ky + 32 : 2, c0 : c0 + 16],
                                start=(n == 0),
                                stop=(n == 17),
                            )
                            n += 1
                    h3 = h3_p.tile([128, 2, 16, 16], FP, name="h3scr")
                    # accumulation runs in fp32 internally; only the final
                    # write is f32r-rounded (it feeds a tf32 matmul anyway)
                    with nc.allow_low_precision(reason="pool feeds f32r matmul"):
                        nc.scalar.activation(
                            out=h3[:, 0, :, :],
                            in_=ps[:, 0, :, :],
                            func=RELU,
                            bias=b3sb[:, mt : mt + 1],
                            accum_out=hpool[:, mt, 2 * pair : 2 * pair + 1],
                        )
                        nc.vector.tensor_scalar(
                            out=h3[:, 1, :, :], in0=ps[:, 1, :, :],
                            scalar1=b3sb[:, mt : mt + 1], scalar2=0.0,
                            op0=ADD, op1=MAX,
                        )
                        nc.vector.tensor_reduce(
                            out=hpool[:, mt, 2 * pair + 1 : 2 * pair + 2],
                            in_=h3[:, 1, :, :],
                            axis=mybir.AxisListType.XY,
                            op=ADD,
                        )
                    if fc1_hook is not None:
                        fc1_hook(mt)

            # conv3(pair p) is sandwiched between conv1 and conv2 of image
            # 2p+2: conv1's matmuls cover the latency of the last conv2
            # drains conv3 depends on, and conv3's long stretch covers
            # conv1's drains that conv2 depends on.
            fwall = None
            for img in range(8):
                pair, i = divmod(img, 2)
                h1pair = h1pads[img % 2]
                h2pair = h2pads[pair % 2]
                for half in range(2):
                    conv1_half(img, h1pair, half)
                    if half == 0 and img + 2 < 8:
                        load_xim(img + 2)
                if i == 0 and pair >= 1:
                    conv3(pair - 1, h2pads[(pair - 1) % 2])
                if img == 1:
                    fwall = consts.tile([128, 5224], BF)
                    nc.sync.dma_start(out=fwall[:, :], in_=fwall_d[:, :])
                for half in range(2):
                    conv2_half(img, h1pair, h2pair, i, half)

            # FC1 with batch on partitions: psf[b, j] = fb1[j] + sum_kt
            # hpool[:, kt, b]^T @ fw1t[:, kt, j]. One 512-wide matmul per
            # (kt, half): 4 LDWEIGHTS of hpool instead of 32 of fw1t, so the
            # PE isn't weight-load-bound. Bias lands first via a K=1 matmul
            # (all-ones lhsT). Chunk kt is issued one mt-group late inside
            # conv3(pair 3) so its hpool dependency is long satisfied and
            # only kt=3 remains on the serial tail.
            psfA = ps1_p.tile([8, 512], FP, name="ps_c1")
            psfB = ps1_p.tile([8, 512], FP, name="ps_c1")
            nc.tensor.matmul(
                out=psfA[:, :], lhsT=ones[0:1, :], rhs=fwall[0:1, 4184:4696],
                start=True, stop=False,
            )
            nc.tensor.matmul(
                out=psfB[:, :], lhsT=ones[0:1, :], rhs=fwall[0:1, 4696:5208],
                start=True, stop=False,
            )

            def fc1_chunk(kt):
                nc.tensor.matmul(
                    out=psfA[:, :], lhsT=hpool[:, kt, :],
                    rhs=fwall[:, 1024 * kt : 1024 * kt + 512],
                    start=False, stop=(kt == 3),
                )
                nc.tensor.matmul(
                    out=psfB[:, :], lhsT=hpool[:, kt, :],
                    rhs=fwall[:, 1024 * kt + 512 : 1024 * kt + 1024],
                    start=False, stop=(kt == 3),
                )

            def fc1_hook(mt):
                if mt >= 1:
                    fc1_chunk(mt - 1)

            conv3(3, h2pads[1], fc1_hook=fc1_hook)
            fc1_chunk(3)

            # relu in 256-col chunks alternating engines so the first
            # transpose starts ~350ns after psfA stops instead of 1.2us
            for q in range(4):
                ps_src = psfA if q < 2 else psfB
                off = 256 * (q % 2)
                if q % 2 == 0:
                    nc.scalar.activation(
                        out=z1[:, 256 * q : 256 * q + 256],
                        in_=ps_src[:, off : off + 256], func=RELU,
                    )
                else:
                    nc.vector.tensor_scalar(
                        out=z1[:, 256 * q : 256 * q + 256],
                        in0=ps_src[:, off : off + 256],
                        scalar1=0.0, scalar2=0.0, op0=ADD, op1=MAX,
                    )

            # z1 [8, 1024] -> z1T [128, 8, 8] via PE transpose (identity rhs),
            # drained by relu (idempotent) alternating engines; FC2 accumulates
            # over the 8 column chunks.
            psf2 = ps1_p.tile([128, 8], FP, name="ps_c1")
            nc.tensor.matmul(
                out=psf2[0:10, :], lhsT=fwall[0:1, 5208:5218], rhs=ones[0:1, :],
                start=True, stop=False,
            )
            for c in range(8):
                zps = (ps23_p if c % 2 == 0 else ps3_p).tile(
                    [128, 8], FP, name="ps_c2" if c % 2 == 0 else "ps_c3"
                )
                nc.tensor.matmul(
                    out=zps[:, :], lhsT=z1[:, 128 * c : 128 * c + 128],
                    rhs=fwall[0:8, 4176:4184], start=True, stop=True,
                )
                if c % 2 == 0:
                    nc.scalar.activation(out=z1T[:, c, :], in_=zps[:, :], func=RELU)
                else:
                    nc.vector.tensor_scalar(
                        out=z1T[:, c, :], in0=zps[:, :], scalar1=0.0, scalar2=0.0,
                        op0=ADD, op1=MAX,
                    )
                nc.tensor.matmul(
                    out=psf2[0:10, :],
                    lhsT=fwall[:, 4096 + 10 * c : 4096 + 10 * c + 10],
                    rhs=z1T[:, c, :],
                    start=False,
                    stop=(c == 7),
                )
            nc.scalar.activation(out=y_sb[:, :], in_=psf2[0:10, :], func=mybir.ActivationFunctionType.Copy)
            nc.sync.dma_start(out=outT_d[:, :], in_=y_sb[:, :])

    nc.compile()
    return nc


def _get_nc():
    if "nc" not in _cache:
        _cache["nc"] = _build()
    return _cache["nc"]


def kernel(**inputs):
    from concourse import bass_utils

    nc = _get_nc()
    xpad, weights = _prep(inputs)
    in_maps = [
        dict(weights, xpad=np.ascontiguousarray(xpad[8 * c : 8 * c + 8]))
        for c in range(8)
    ]
    res = bass_utils.run_bass_kernel_spmd(
        nc, in_maps, core_ids=list(range(8)), trace=TRACE
    )
    LAST["exec_time_ns"] = getattr(res, "exec_time_ns", None)
    LAST["profile_json"] = getattr(res, "profile_json", None)
    LAST["instructions_and_trace"] = getattr(res, "instructions_and_trace", None)
    out = np.concatenate([r["outT"].T for r in res.results], axis=0)
    return np.ascontiguousarray(out.astype(np.float32))


# revision 48
# speedup vs baseline: 1.0336x; 1.0120x over previous
import sys

import numpy as np
from ml_dtypes import bfloat16

sys.path.insert(0, "/opt/trn_rl_repo")

TRACE = False
LAST = {}
_cache = {}

SPARSITY = 0.5

# even columns first, then odd: makes the stride-2 convs read contiguously
_XPERM = np.r_[1:64:2, 0:64:2]


def _tf32(a):
    b = np.ascontiguousarray(np.asarray(a, np.float32))
    u = b.view(np.uint32).copy()
    u += np.uint32(0x0FFF) + ((u >> np.uint32(13)) & np.uint32(1))
    u &= np.uint32(0xFFFFE000)
    return u.view(np.float32)


def _masked(w, s):
    sa = np.abs(np.asarray(s, np.float32)).ravel()
    j = int((1.0 - SPARSITY) * sa.size)
    thr = np.partition(sa, j)[j]
    m = (np.abs(np.asarray(s, np.float32)) >= thr).astype(np.float32)
    return (np.asarray(w, np.float32) * m).astype(np.float32)


def _prep(inputs):
    w1m = _masked(inputs["w1"], inputs["s1"])  # [128,3,3,3]
    w2m = _masked(inputs["w2"], inputs["s2"])  # [256,128,3,3]
    w3m = _masked(inputs["w3"], inputs["s3"])  # [512,256,3,3]
    fw1m = _masked(inputs["fw1"], inputs["fs1"])  # [1024,512]
    fw2m = _masked(inputs["fw2"], inputs["fs2"])  # [10,1024]

    c = np.ascontiguousarray
    # conv1 as single K=27 matmul, K padded to 32 and replicated 4x across
    # partition groups so rhs tiles at base partitions 0/32/64/96 line up
    w1t = np.zeros((64, 128), np.float32)
    w1t[:27] = w1m.transpose(1, 2, 3, 0).reshape(27, 128)
    w1t[32:59] = w1t[:27]
    w2t = c(w2m.transpose(1, 2, 3, 0).reshape(128, 9 * 256))
    # mt-major: [k2, mt, kt*1152 + g*128 + o]
    w3t = c(
        w3m.reshape(4, 128, 2, 128, 3, 3)
        .transpose(3, 0, 2, 4, 5, 1)
        .reshape(128, 4, 2304)
    )
    # global-avg-pool 1/256 folded into fw1
    fw1t = c((fw1m.T.reshape(4, 128, 1024).transpose(1, 0, 2) / 256.0).astype(np.float32))
    fw2t = c(fw2m.T.reshape(8, 128, 10).transpose(1, 0, 2))

    weights = {
        "w1t": w1t.astype(bfloat16),
        "w2t": w2t.astype(bfloat16),
        "w3t": w3t.astype(bfloat16),
        "fwall": None,  # filled below
        "bias": np.concatenate(
            [
                np.asarray(inputs["b1"], np.float32).reshape(128, 1),
                np.asarray(inputs["b2"], np.float32).reshape(2, 128).T,
                np.asarray(inputs["b3"], np.float32).reshape(4, 128).T,
            ],
            axis=1,
        ),
    }
    fwall = np.zeros((128, 5224), np.float32)
    fwall[:, 0:4096] = fw1t.reshape(128, 4096)
    fwall[:, 4096:4176] = fw2t.reshape(128, 80)
    fwall[0:8, 4176:4184] = np.eye(8, dtype=np.float32)
    fwall[0:1, 4184:5208] = np.asarray(inputs["fb1"], np.float32).reshape(1, 1024)
    fwall[0:1, 5208:5218] = np.asarray(inputs["fb2"], np.float32).reshape(1, 10)
    weights["fwall"] = fwall.astype(bfloat16)
    xpad = np.zeros((64, 3, 66, 66), np.float32)
    xpad[:, :, 1:65, 1:65] = np.asarray(inputs["x"], np.float32)
    # im2col over (ch,ky,kx): x27[i, ch*9+ky*3+kx] = xpad[i, ch, ky:ky+64, kx:kx+64]
    x27 = np.empty((64, 27, 64, 64), np.float32)
    for ch in range(3):
        for ky in range(3):
            for kx in range(3):
                x27[:, ch * 9 + ky * 3 + kx] = xpad[:, ch, ky : ky + 64, kx : kx + 64]
    # pack for full-width DMA + phase-split columns:
    # xim32[i, half, 32*nt + k, r, px] = x27[i, k, 32*half + 8*nt + r, XPERM[px]]
    xr = x27[:, :, :, _XPERM].reshape(64, 27, 2, 4, 8, 64)  # [i,k,half,nt,r,px]
    # partition p = 32*a + k holds nt = 2*b + a of half h at free slot 2h+b:
    # matmul rhs bases stay at 0/32, one whole-image DMA per image.
    xim32 = np.zeros((64, 2, 32, 4, 8, 64), np.float32)  # [i,a,k,2h+b,r,px]
    for a in range(2):
        for h in range(2):
            for b in range(2):
                xim32[:, a, :27, 2 * h + b] = xr[:, :, h, 2 * b + a]
    xim32 = c(xim32.reshape(64, 64, 4, 8, 64).astype(bfloat16))
    return xim32, weights


def _build():
    import concourse.bacc as bacc
    import concourse.mybir as mybir
    import concourse.tile as tile

    FP = mybir.dt.float32
    FR = mybir.dt.float32r
    BF = mybir.dt.bfloat16
    RELU = mybir.ActivationFunctionType.Relu
    ADD = mybir.AluOpType.add
    MAX = mybir.AluOpType.max

    nc = bacc.Bacc("TRN2", target_bir_lowering=False, debug=False)

    xpad_d = nc.dram_tensor("xpad", [8, 64, 4, 8, 64], BF, kind="ExternalInput")
    w1t_d = nc.dram_tensor("w1t", [64, 128], BF, kind="ExternalInput")
    w2t_d = nc.dram_tensor("w2t", [128, 2304], BF, kind="ExternalInput")
    w3t_d = nc.dram_tensor("w3t", [128, 4, 2304], BF, kind="ExternalInput")
    fwall_d = nc.dram_tensor("fwall", [128, 5224], BF, kind="ExternalInput")
    bias_d = nc.dram_tensor("bias", [128, 7], FP, kind="ExternalInput")
    outT_d = nc.dram_tensor("outT", [10, 8], FP, kind="ExternalOutput")

    with tile.TileContext(nc) as tc:
        with tc.tile_pool(name="consts", bufs=1) as consts, \
             tc.tile_pool(name="xim_p", bufs=2) as xim_p, \
             tc.tile_pool(name="act_p", bufs=1) as act_p, \
             tc.tile_pool(name="h3_p", bufs=2) as h3_p, \
             tc.tile_pool(name="ps1_p", bufs=4, space="PSUM") as ps1_p, \
             tc.tile_pool(name="ps23_p", bufs=2, space="PSUM") as ps23_p, \
             tc.tile_pool(name="ps3_p", bufs=2, space="PSUM") as ps3_p:

            xims = {}

            def load_xim(img):
                t = xim_p.tile([64, 4, 8, 64], BF, name="ximg")
                nc.sync.dma_start(out=t[:, :, :, :], in_=xpad_d[img, :, :, :, :])
                xims[img] = t

            # DMA issue order = global transfer order: descriptor generation is
            # serialized (~0.7us each) on the Sync queue, so the tensors the
            # first matmuls need go first. Image 0 streams as two halves.
            xim00 = xim_p.tile([64, 2, 8, 64], BF, name="ximg")
            nc.sync.dma_start(out=xim00[:, :, :, :], in_=xpad_d[0, :, 0:2, :, :])
            w1t = consts.tile([64, 128], BF)
            nc.sync.dma_start(out=w1t[:, :], in_=w1t_d[:, :])
            xim01 = xim_p.tile([64, 2, 8, 64], BF, name="ximg")
            nc.sync.dma_start(out=xim01[:, :, :, :], in_=xpad_d[0, :, 2:4, :, :])
            xims[(0, 0)], xims[(0, 1)] = xim00, xim01
            bias = consts.tile([128, 7], FP)
            nc.sync.dma_start(out=bias[:, :], in_=bias_d[:, :])
            w2t = consts.tile([128, 2304], BF)
            nc.sync.dma_start(out=w2t[:, :], in_=w2t_d[:, :])
            load_xim(1)
            w3all = consts.tile([128, 4, 2304], BF)
            nc.sync.dma_start(out=w3all[:, :, :], in_=w3t_d[:, :, :])
            b1sb = bias[:, 0:1]
            b2sb = bias[:, 1:3]
            b3sb = bias[:, 3:7]

            # PE p-state warm-up: ramp runs on wall time since first dispatch,
            # so a burst of throwaway matmuls during the DMA wait gets the
            # engine to speed before conv1 starts.
            warm = consts.tile([32, 8, 64], BF)
            nc.vector.memset(warm[:, :, :], 0.0)
            ones = consts.tile([1, 8], BF)
            nc.vector.memset(ones[:, :], 1.0)
            for _ in range(4):
                wps = ps1_p.tile([128, 8, 64], FP, name="ps_c1")
                nc.tensor.matmul(
                    out=wps[0:64, :, :], lhsT=warm[:, 0, :], rhs=warm[:, :, :],
                    start=True, stop=True,
                )

            # h1 phase-split in one tile: cols 0:33 = even input cols
            # (0,2,..,64), cols 33:66 = odd (1,3,..,65). With the odds-first
            # pixel order from the host, a conv1 PSUM tile drains to
            # h1eo[:, rows, 1:65] in a single contiguous op.
            h1eo_a = act_p.tile([128, 66, 66], BF)
            h1eo_b = act_p.tile([128, 66, 66], BF)
            # h2 phase-split: h2e = cols 0,2,..,32 (17), h2o = cols 1,..,33 (17)
            h2e_a = act_p.tile([128, 2, 2, 34, 17], BF)
            h2o_a = act_p.tile([128, 2, 2, 34, 17], BF)
            h2e_b = act_p.tile([128, 2, 2, 34, 17], BF)
            h2o_b = act_p.tile([128, 2, 2, 34, 17], BF)
            hpool = act_p.tile([128, 4, 8], BF)
            z1 = act_p.tile([8, 1024], BF)
            z1T = act_p.tile([128, 8, 8], BF)
            y_sb = act_p.tile([10, 8], FP)

            # Border-only zeroing: interiors are fully overwritten every image,
            # borders stay zero for the kernel's lifetime.
            for h1eo in (h1eo_a, h1eo_b):
                nc.vector.memset(h1eo[:, 0, :], 0.0)
                nc.vector.memset(h1eo[:, 65, :], 0.0)
                nc.vector.memset(h1eo[:, 1:65, 0], 0.0)
                nc.vector.memset(h1eo[:, 1:65, 65], 0.0)
            for h2e, h2o in ((h2e_a, h2o_a), (h2e_b, h2o_b)):
                for m in range(2):
                    for i in range(2):
                        nc.vector.memset(h2e[:, m, i, 0, :], 0.0)
                        nc.vector.memset(h2e[:, m, i, 33, :], 0.0)
                        nc.vector.memset(h2e[:, m, i, 1:33, 0], 0.0)
                        nc.vector.memset(h2o[:, m, i, 0, :], 0.0)
                        nc.vector.memset(h2o[:, m, i, 33, :], 0.0)
                        nc.vector.memset(h2o[:, m, i, 1:33, 16], 0.0)

            h1pads = [h1eo_a, h1eo_b]
            h2pads = [(h2e_a, h2o_a), (h2e_b, h2o_b)]

            def drain(eng, out, ps, bias):
                if eng == 0:
                    nc.scalar.activation(out=out, in_=ps, func=RELU, bias=bias)
                else:
                    nc.vector.tensor_scalar(
                        out=out, in0=ps, scalar1=bias, scalar2=0.0,
                        op0=ADD, op1=MAX,
                    )

            def conv1_half(img, h1eo, half):
                if img == 0:
                    xim = xims.pop((0, half))
                    slot = 0
                else:
                    xim = xims.pop(img) if half == 1 else xims[img]
                    slot = 2 * half
                for nt in range(4):
                    ntg = 4 * half + nt
                    ps = ps1_p.tile([128, 8, 64], FP, name="ps_c1")
                    nc.tensor.matmul(
                        out=ps[:, :, :],
                        lhsT=w1t[32 * (nt % 2) : 32 * (nt % 2) + 32, :],
                        rhs=xim[32 * (nt % 2) : 32 * (nt % 2) + 32, slot + nt // 2, :, :],
                        start=True,
                        stop=True,
                    )
                    # odds-first pixel order: ps col j -> h1eo col j+1;
                    # split across both engines to halve the PSUM WAR latency
                    r0 = 1 + 8 * ntg
                    drain(ntg % 2, h1eo[:, r0 : r0 + 8, 1:33],
                          ps[:, :, 0:32], b1sb[:, 0:1])
                    drain(1 - ntg % 2, h1eo[:, r0 : r0 + 8, 33:65],
                          ps[:, :, 32:64], b1sb[:, 0:1])

            def conv2_half(img, h1eo, h2pair, islot, nh):
                h2e, h2o = h2pair
                # g-outer for rhs reuse; m=0 runs its last two passes early
                # so its drains overlap m=1's final matmuls and its PSUM slot
                # is free when the next half starts
                pss = [ps23_p.tile([128, 16, 32], FP, name="ps_c2") for _ in range(2)]
                # m1 trails m0 by 3 passes: the half opens m0-only (covering
                # the previous half's m1 drains) and closes m1-only (m0's
                # drains overlap), so neither PSUM slot is hot at a boundary
                order = [(0, 0), (1, 0), (2, 0)]
                for g in range(3, 9):
                    order += [(g, 0), (g - 3, 1)]
                order += [(6, 1), (7, 1), (8, 1)]

                def c2drain(m):
                    # out x' even -> h2 odd cols -> h2o[0:16]; odd -> h2e[1:17]
                    r0 = 1 + 16 * nh
                    drain(0 if m == 0 else 1, h2o[:, m, islot, r0 : r0 + 16, 0:16],
                          pss[m][:, :, 0:32:2], b2sb[:, m : m + 1])
                    drain(1 if m == 0 else 0, h2e[:, m, islot, r0 : r0 + 16, 1:17],
                          pss[m][:, :, 1:32:2], b2sb[:, m : m + 1])

                for g, m in order:
                    ky, kx = g // 3, g % 3
                    c0 = 0 if kx == 0 else 33 if kx == 1 else 1
                    r0 = 32 * nh + ky
                    nc.tensor.matmul(
                        out=pss[m][:, :, :],
                        lhsT=w2t[:, 256 * g + 128 * m : 256 * g + 128 * m + 128],
                        rhs=h1eo[:, r0 : r0 + 32 : 2, c0 : c0 + 32],
                        start=(g == 0),
                        stop=(g == 8),
                    )
                    if (g, m) == (8, 0):
                        c2drain(0)
                c2drain(1)

            def conv3(pair, h2pair, fc1_hook=None):
                h2e, h2o = h2pair
                for mt in range(4):
                    ps = ps3_p.tile([128, 2, 16, 16], FP, name="ps_c3")
                    n = 0
                    for kt in range(2):
                        for g in range(9):
                            ky, kx = g // 3, g % 3
                            hsrc, c0 = (
                                (h2e, 0) if kx == 0 else (h2o, 0) if kx == 1 else (h2e, 1)
                            )
                            nc.tensor.matmul(
                                out=ps[:, :, :, :],
                                lhsT=w3all[:, mt, 1152 * kt + 128 * g : 1152 * kt + 128 * g + 128],
                                rhs=hsrc[:, kt, :, ky : ky + 32 : 2, c0 : c0 + 16],
                                start=(n == 0),
                                stop=(n == 17),
                            )
                            n += 1
                    h3 = h3_p.tile([128, 2, 16, 16], FP, name="h3scr")
                    # accumulation runs in fp32 internally; only the final
                    # write is f32r-rounded (it feeds a tf32 matmul anyway)
                    with nc.allow_low_precision(reason="pool feeds f32r matmul"):
                        nc.scalar.activation(
                            out=h3[:, 0, :, :],
                            in_=ps[:, 0, :, :],
                            func=RELU,
                            bias=b3sb[:, mt : mt + 1],
                            accum_out=hpool[:, mt, 2 * pair : 2 * pair + 1],
                        )
                        nc.vector.tensor_scalar(
                            out=h3[:, 1, :, :], in0=ps[:, 1, :, :],
                            scalar1=b3sb[:, mt : mt + 1], scalar2=0.0,
                            op0=ADD, op1=MAX,
                        )
                        nc.vector.tensor_reduce(
                            out=hpool[:, mt, 2 * pair + 1 : 2 * pair + 2],
                            in_=h3[:, 1, :, :],
                            axis=mybir.AxisListType.XY,
                            op=ADD,
                        )
                    if fc1_hook is not None:
                        fc1_hook(mt)

            # conv3(pair p) is sandwiched between conv1 and conv2 of image
            # 2p+2: conv1's matmuls cover the latency of the last conv2
            # drains conv3 depends on, and conv3's long stretch covers
            # conv1's drains that conv2 depends on.
            fwall = None
            for img in range(8):
                pair, i = divmod(img, 2)
                h1pair = h1pads[img % 2]
                h2pair = h2pads[pair % 2]
                for half in range(2):
                    conv1_half(img, h1pair, half)
                    if half == 0 and img + 2 < 8:
                        load_xim(img + 2)
                if i == 0 and pair >= 1:
                    conv3(pair - 1, h2pads[(pair - 1) % 2])
                if img == 1:
                    fwall = consts.tile([128, 5224], BF)
                    nc.sync.dma_start(out=fwall[:, :], in_=fwall_d[:, :])
                if i == 0:
                    # defer this image's second conv2 half: it runs after the
                    # NEXT image's conv1, covering those drains' latency
                    conv2_half(img, h1pair, h2pair, i, 0)
                else:
                    conv2_half(img - 1, h1pads[(img - 1) % 2], h2pair, 0, 1)
                    conv2_half(img, h1pair, h2pair, i, 0)
                    conv2_half(img, h1pair, h2pair, i, 1)

            # FC1 with batch on partitions: psf[b, j] = fb1[j] + sum_kt
            # hpool[:, kt, b]^T @ fw1t[:, kt, j]. One 512-wide matmul per
            # (kt, half): 4 LDWEIGHTS of hpool instead of 32 of fw1t, so the
            # PE isn't weight-load-bound. Bias lands first via a K=1 matmul
            # (all-ones lhsT). Chunk kt is issued one mt-group late inside
            # conv3(pair 3) so its hpool dependency is long satisfied and
            # only kt=3 remains on the serial tail.
            psfA = ps1_p.tile([8, 512], FP, name="ps_c1")
            psfB = ps1_p.tile([8, 512], FP, name="ps_c1")
            nc.tensor.matmul(
                out=psfA[:, :], lhsT=ones[0:1, :], rhs=fwall[0:1, 4184:4696],
                start=True, stop=False,
            )
            nc.tensor.matmul(
                out=psfB[:, :], lhsT=ones[0:1, :], rhs=fwall[0:1, 4696:5208],
                start=True, stop=False,
            )

            def fc1_chunk(kt):
                nc.tensor.matmul(
                    out=psfA[:, :], lhsT=hpool[:, kt, :],
                    rhs=fwall[:, 1024 * kt : 1024 * kt + 512],
                    start=False, stop=(kt == 3),
                )
                nc.tensor.matmul(
                    out=psfB[:, :], lhsT=hpool[:, kt, :],
                    rhs=fwall[:, 1024 * kt + 512 : 1024 * kt + 1024],
                    start=False, stop=(kt == 3),
                )

            def fc1_hook(mt):
                if mt >= 1:
                    fc1_chunk(mt - 1)

            conv3(3, h2pads[1], fc1_hook=fc1_hook)
            fc1_chunk(3)

            # relu in 256-col chunks alternating engines so the first
            # transpose starts ~350ns after psfA stops instead of 1.2us
            for q in range(4):
                ps_src = psfA if q < 2 else psfB
                off = 256 * (q % 2)
                if q % 2 == 0:
                    nc.scalar.activation(
                        out=z1[:, 256 * q : 256 * q + 256],
                        in_=ps_src[:, off : off + 256], func=RELU,
                    )
                else:
                    nc.vector.tensor_scalar(
                        out=z1[:, 256 * q : 256 * q + 256],
                        in0=ps_src[:, off : off + 256],
                        scalar1=0.0, scalar2=0.0, op0=ADD, op1=MAX,
                    )

            # z1 [8, 1024] -> z1T [128, 8, 8] via PE transpose (identity rhs),
            # drained by relu (idempotent) alternating engines; FC2 accumulates
            # over the 8 column chunks.
            psf2 = ps1_p.tile([128, 8], FP, name="ps_c1")
            nc.tensor.matmul(
                out=psf2[0:10, :], lhsT=fwall[0:1, 5208:5218], rhs=ones[0:1, :],
                start=True, stop=False,
            )
            for c in range(8):
                zps = (ps23_p if c % 2 == 0 else ps3_p).tile(
                    [128, 8], FP, name="ps_c2" if c % 2 == 0 else "ps_c3"
                )
                nc.tensor.matmul(
                    out=zps[:, :], lhsT=z1[:, 128 * c : 128 * c + 128],
                    rhs=fwall[0:8, 4176:4184], start=True, stop=True,
                )
                if c % 2 == 0:
                    nc.scalar.activation(out=z1T[:, c, :], in_=zps[:, :], func=RELU)
                else:
                    nc.vector.tensor_scalar(
                        out=z1T[:, c, :], in0=zps[:, :], scalar1=0.0, scalar2=0.0,
                        op0=ADD, op1=MAX,
                    )
                nc.tensor.matmul(
                    out=psf2[0:10, :],
                    lhsT=fwall[:, 4096 + 10 * c : 4096 + 10 * c + 10],
                    rhs=z1T[:, c, :],
                    start=False,
                    stop=(c == 7),
                )
            nc.scalar.activation(out=y_sb[:, :], in_=psf2[0:10, :], func=mybir.ActivationFunctionType.Copy)
            nc.sync.dma_start(out=outT_d[:, :], in_=y_sb[:, :])

    nc.compile()
    return nc


def _get_nc():
    if "nc" not in _cache:
        _cache["nc"] = _build()
    return _cache["nc"]


def kernel(**inputs):
    from concourse import bass_utils

    nc = _get_nc()
    xpad, weights = _prep(inputs)
    in_maps = [
        dict(weights, xpad=np.ascontiguousarray(xpad[8 * c : 8 * c + 8]))
        for c in range(8)
    ]
    res = bass_utils.run_bass_kernel_spmd(
        nc, in_maps, core_ids=list(range(8)), trace=TRACE
    )
    LAST["exec_time_ns"] = getattr(res, "exec_time_ns", None)
    LAST["profile_json"] = getattr(res, "profile_json", None)
    LAST["instructions_and_trace"] = getattr(res, "instructions_and_trace", None)
    out = np.concatenate([r["outT"].T for r in res.results], axis=0)
    return np.ascontiguousarray(out.astype(np.float32))


# revision 50
# speedup vs baseline: 1.0355x; 1.0018x over previous
import sys

import numpy as np
from ml_dtypes import bfloat16

sys.path.insert(0, "/opt/trn_rl_repo")

TRACE = False
LAST = {}
_cache = {}

SPARSITY = 0.5

# even columns first, then odd: makes the stride-2 convs read contiguously
_XPERM = np.r_[1:64:2, 0:64:2]


def _tf32(a):
    b = np.ascontiguousarray(np.asarray(a, np.float32))
    u = b.view(np.uint32).copy()
    u += np.uint32(0x0FFF) + ((u >> np.uint32(13)) & np.uint32(1))
    u &= np.uint32(0xFFFFE000)
    return u.view(np.float32)


def _masked(w, s):
    sa = np.abs(np.asarray(s, np.float32)).ravel()
    j = int((1.0 - SPARSITY) * sa.size)
    thr = np.partition(sa, j)[j]
    m = (np.abs(np.asarray(s, np.float32)) >= thr).astype(np.float32)
    return (np.asarray(w, np.float32) * m).astype(np.float32)


def _prep(inputs):
    w1m = _masked(inputs["w1"], inputs["s1"])  # [128,3,3,3]
    w2m = _masked(inputs["w2"], inputs["s2"])  # [256,128,3,3]
    w3m = _masked(inputs["w3"], inputs["s3"])  # [512,256,3,3]
    fw1m = _masked(inputs["fw1"], inputs["fs1"])  # [1024,512]
    fw2m = _masked(inputs["fw2"], inputs["fs2"])  # [10,1024]

    c = np.ascontiguousarray
    # conv1 as single K=27 matmul, K padded to 32 and replicated 4x across
    # partition groups so rhs tiles at base partitions 0/32/64/96 line up
    w1t = np.zeros((64, 128), np.float32)
    w1t[:27] = w1m.transpose(1, 2, 3, 0).reshape(27, 128)
    w1t[32:59] = w1t[:27]
    w2t = c(w2m.transpose(1, 2, 3, 0).reshape(128, 9 * 256))
    # mt-major: [k2, mt, kt*1152 + g*128 + o]
    w3t = c(
        w3m.reshape(4, 128, 2, 128, 3, 3)
        .transpose(3, 0, 2, 4, 5, 1)
        .reshape(128, 4, 2304)
    )
    # global-avg-pool 1/256 folded into fw1
    fw1t = c((fw1m.T.reshape(4, 128, 1024).transpose(1, 0, 2) / 256.0).astype(np.float32))
    fw2t = c(fw2m.T.reshape(8, 128, 10).transpose(1, 0, 2))

    weights = {
        "w1t": w1t.astype(bfloat16),
        "w2t": w2t.astype(bfloat16),
        "w3t": w3t.astype(bfloat16),
        "fwall": None,  # filled below
        "bias": np.concatenate(
            [
                np.asarray(inputs["b1"], np.float32).reshape(128, 1),
                np.asarray(inputs["b2"], np.float32).reshape(2, 128).T,
                np.asarray(inputs["b3"], np.float32).reshape(4, 128).T,
            ],
            axis=1,
        ),
    }
    fwall = np.zeros((128, 5224), np.float32)
    fwall[:, 0:4096] = fw1t.reshape(128, 4096)
    fwall[:, 4096:4176] = fw2t.reshape(128, 80)
    fwall[0:8, 4176:4184] = np.eye(8, dtype=np.float32)
    fwall[0:1, 4184:5208] = np.asarray(inputs["fb1"], np.float32).reshape(1, 1024)
    fwall[0:1, 5208:5218] = np.asarray(inputs["fb2"], np.float32).reshape(1, 10)
    weights["fwall"] = fwall.astype(bfloat16)
    xpad = np.zeros((64, 3, 66, 66), np.float32)
    xpad[:, :, 1:65, 1:65] = np.asarray(inputs["x"], np.float32)
    # im2col over (ch,ky,kx): x27[i, ch*9+ky*3+kx] = xpad[i, ch, ky:ky+64, kx:kx+64]
    x27 = np.empty((64, 27, 64, 64), np.float32)
    for ch in range(3):
        for ky in range(3):
            for kx in range(3):
                x27[:, ch * 9 + ky * 3 + kx] = xpad[:, ch, ky : ky + 64, kx : kx + 64]
    # pack for full-width DMA + phase-split columns:
    # xim32[i, half, 32*nt + k, r, px] = x27[i, k, 32*half + 8*nt + r, XPERM[px]]
    xr = x27[:, :, :, _XPERM].reshape(64, 27, 2, 4, 8, 64)  # [i,k,half,nt,r,px]
    # partition p = 32*a + k holds nt = 2*b + a of half h at free slot 2h+b:
    # matmul rhs bases stay at 0/32, one whole-image DMA per image.
    xim32 = np.zeros((64, 2, 32, 4, 8, 64), np.float32)  # [i,a,k,2h+b,r,px]
    for a in range(2):
        for h in range(2):
            for b in range(2):
                xim32[:, a, :27, 2 * h + b] = xr[:, :, h, 2 * b + a]
    xim32 = c(xim32.reshape(64, 64, 4, 8, 64).astype(bfloat16))
    return xim32, weights


def _build():
    import concourse.bacc as bacc
    import concourse.mybir as mybir
    import concourse.tile as tile

    FP = mybir.dt.float32
    FR = mybir.dt.float32r
    BF = mybir.dt.bfloat16
    RELU = mybir.ActivationFunctionType.Relu
    ADD = mybir.AluOpType.add
    MAX = mybir.AluOpType.max

    nc = bacc.Bacc("TRN2", target_bir_lowering=False, debug=False)

    xpad_d = nc.dram_tensor("xpad", [8, 64, 4, 8, 64], BF, kind="ExternalInput")
    w1t_d = nc.dram_tensor("w1t", [64, 128], BF, kind="ExternalInput")
    w2t_d = nc.dram_tensor("w2t", [128, 2304], BF, kind="ExternalInput")
    w3t_d = nc.dram_tensor("w3t", [128, 4, 2304], BF, kind="ExternalInput")
    fwall_d = nc.dram_tensor("fwall", [128, 5224], BF, kind="ExternalInput")
    bias_d = nc.dram_tensor("bias", [128, 7], FP, kind="ExternalInput")
    outT_d = nc.dram_tensor("outT", [10, 8], FP, kind="ExternalOutput")

    with tile.TileContext(nc) as tc:
        with tc.tile_pool(name="consts", bufs=1) as consts, \
             tc.tile_pool(name="xim_p", bufs=2) as xim_p, \
             tc.tile_pool(name="act_p", bufs=1) as act_p, \
             tc.tile_pool(name="h3_p", bufs=2) as h3_p, \
             tc.tile_pool(name="ps1_p", bufs=4, space="PSUM") as ps1_p, \
             tc.tile_pool(name="ps23_p", bufs=2, space="PSUM") as ps23_p, \
             tc.tile_pool(name="ps3_p", bufs=2, space="PSUM") as ps3_p:

            xims = {}

            def load_xim(img):
                t = xim_p.tile([64, 4, 8, 64], BF, name="ximg")
                nc.sync.dma_start(out=t[:, :, :, :], in_=xpad_d[img, :, :, :, :])
                xims[img] = t

            # DMA issue order = global transfer order: descriptor generation is
            # serialized (~0.7us each) on the Sync queue, so the tensors the
            # first matmuls need go first. Image 0 streams as two halves.
            xim00 = xim_p.tile([64, 2, 8, 64], BF, name="ximg")
            nc.sync.dma_start(out=xim00[:, :, :, :], in_=xpad_d[0, :, 0:2, :, :])
            w1t = consts.tile([64, 128], BF)
            nc.sync.dma_start(out=w1t[:, :], in_=w1t_d[:, :])
            xim01 = xim_p.tile([64, 2, 8, 64], BF, name="ximg")
            nc.sync.dma_start(out=xim01[:, :, :, :], in_=xpad_d[0, :, 2:4, :, :])
            xims[(0, 0)], xims[(0, 1)] = xim00, xim01
            bias = consts.tile([128, 7], FP)
            nc.sync.dma_start(out=bias[:, :], in_=bias_d[:, :])
            w2t = consts.tile([128, 2304], BF)
            nc.sync.dma_start(out=w2t[:, :], in_=w2t_d[:, :])
            load_xim(1)
            w3all = consts.tile([128, 4, 2304], BF)
            nc.sync.dma_start(out=w3all[:, :, :], in_=w3t_d[:, :, :])
            b1sb = bias[:, 0:1]
            b2sb = bias[:, 1:3]
            b3sb = bias[:, 3:7]

            # PE p-state warm-up: ramp runs on wall time since first dispatch,
            # so a burst of throwaway matmuls during the DMA wait gets the
            # engine to speed before conv1 starts.
            warm = consts.tile([32, 8, 64], BF)
            nc.vector.memset(warm[:, :, :], 0.0)
            ones = consts.tile([1, 8], BF)
            nc.vector.memset(ones[:, :], 1.0)
            for _ in range(4):
                wps = ps1_p.tile([128, 8, 64], FP, name="ps_c1")
                nc.tensor.matmul(
                    out=wps[0:64, :, :], lhsT=warm[:, 0, :], rhs=warm[:, :, :],
                    start=True, stop=True,
                )

            # h1 phase-split in one tile: cols 0:33 = even input cols
            # (0,2,..,64), cols 33:66 = odd (1,3,..,65). With the odds-first
            # pixel order from the host, a conv1 PSUM tile drains to
            # h1eo[:, rows, 1:65] in a single contiguous op.
            h1eo_a = act_p.tile([128, 66, 66], BF)
            h1eo_b = act_p.tile([128, 66, 66], BF)
            # h2 phase-split: h2e = cols 0,2,..,32 (17), h2o = cols 1,..,33 (17)
            h2e_a = act_p.tile([128, 2, 2, 34, 17], BF)
            h2o_a = act_p.tile([128, 2, 2, 34, 17], BF)
            h2e_b = act_p.tile([128, 2, 2, 34, 17], BF)
            h2o_b = act_p.tile([128, 2, 2, 34, 17], BF)
            hpool = act_p.tile([128, 4, 8], BF)
            z1 = act_p.tile([8, 1024], BF)
            z1T = act_p.tile([128, 8, 8], BF)
            y_sb = act_p.tile([10, 8], FP)

            # Border-only zeroing: interiors are fully overwritten every image,
            # borders stay zero for the kernel's lifetime.
            for h1eo in (h1eo_a, h1eo_b):
                nc.vector.memset(h1eo[:, 0, :], 0.0)
                nc.vector.memset(h1eo[:, 65, :], 0.0)
                nc.vector.memset(h1eo[:, 1:65, 0], 0.0)
                nc.vector.memset(h1eo[:, 1:65, 65], 0.0)
            for h2e, h2o in ((h2e_a, h2o_a), (h2e_b, h2o_b)):
                for m in range(2):
                    for i in range(2):
                        nc.vector.memset(h2e[:, m, i, 0, :], 0.0)
                        nc.vector.memset(h2e[:, m, i, 33, :], 0.0)
                        nc.vector.memset(h2e[:, m, i, 1:33, 0], 0.0)
                        nc.vector.memset(h2o[:, m, i, 0, :], 0.0)
                        nc.vector.memset(h2o[:, m, i, 33, :], 0.0)
                        nc.vector.memset(h2o[:, m, i, 1:33, 16], 0.0)

            h1pads = [h1eo_a, h1eo_b]
            h2pads = [(h2e_a, h2o_a), (h2e_b, h2o_b)]

            def drain(eng, out, ps, bias):
                if eng == 0:
                    nc.scalar.activation(out=out, in_=ps, func=RELU, bias=bias)
                else:
                    nc.vector.tensor_scalar(
                        out=out, in0=ps, scalar1=bias, scalar2=0.0,
                        op0=ADD, op1=MAX,
                    )

            def conv1_half(img, h1eo, half):
                if img == 0:
                    xim = xims.pop((0, half))
                    slot = 0
                else:
                    xim = xims.pop(img) if half == 1 else xims[img]
                    slot = 2 * half
                for nt in range(4):
                    ntg = 4 * half + nt
                    ps = ps1_p.tile([128, 8, 64], FP, name="ps_c1")
                    nc.tensor.matmul(
                        out=ps[:, :, :],
                        lhsT=w1t[32 * (nt % 2) : 32 * (nt % 2) + 32, :],
                        rhs=xim[32 * (nt % 2) : 32 * (nt % 2) + 32, slot + nt // 2, :, :],
                        start=True,
                        stop=True,
                    )
                    # odds-first pixel order: ps col j -> h1eo col j+1;
                    # split across both engines to halve the PSUM WAR latency
                    r0 = 1 + 8 * ntg
                    drain(ntg % 2, h1eo[:, r0 : r0 + 8, 1:33],
                          ps[:, :, 0:32], b1sb[:, 0:1])
                    drain(1 - ntg % 2, h1eo[:, r0 : r0 + 8, 33:65],
                          ps[:, :, 32:64], b1sb[:, 0:1])

            def conv2_half(img, h1eo, h2pair, islot, nh):
                h2e, h2o = h2pair
                # g-outer for rhs reuse; m=0 runs its last two passes early
                # so its drains overlap m=1's final matmuls and its PSUM slot
                # is free when the next half starts
                pss = [ps23_p.tile([128, 16, 32], FP, name="ps_c2") for _ in range(2)]
                # m1 trails m0 by 3 passes: the half opens m0-only (covering
                # the previous half's m1 drains) and closes m1-only (m0's
                # drains overlap), so neither PSUM slot is hot at a boundary
                order = [(0, 0), (1, 0), (2, 0)]
                for g in range(3, 9):
                    order += [(g, 0), (g - 3, 1)]
                order += [(6, 1), (7, 1), (8, 1)]

                def c2drain(m):
                    # out x' even -> h2 odd cols -> h2o[0:16]; odd -> h2e[1:17]
                    r0 = 1 + 16 * nh
                    drain(0 if m == 0 else 1, h2o[:, m, islot, r0 : r0 + 16, 0:16],
                          pss[m][:, :, 0:32:2], b2sb[:, m : m + 1])
                    drain(1 if m == 0 else 0, h2e[:, m, islot, r0 : r0 + 16, 1:17],
                          pss[m][:, :, 1:32:2], b2sb[:, m : m + 1])

                for g, m in order:
                    ky, kx = g // 3, g % 3
                    c0 = 0 if kx == 0 else 33 if kx == 1 else 1
                    r0 = 32 * nh + ky
                    nc.tensor.matmul(
                        out=pss[m][:, :, :],
                        lhsT=w2t[:, 256 * g + 128 * m : 256 * g + 128 * m + 128],
                        rhs=h1eo[:, r0 : r0 + 32 : 2, c0 : c0 + 32],
                        start=(g == 0),
                        stop=(g == 8),
                    )
                    if (g, m) == (8, 0):
                        c2drain(0)
                c2drain(1)

            def conv3(pair, h2pair, fc1_hook=None):
                h2e, h2o = h2pair
                for mt in range(4):
                    ps = ps3_p.tile([128, 2, 16, 16], FP, name="ps_c3")
                    n = 0
                    for kt in range(2):
                        for g in range(9):
                            ky, kx = g // 3, g % 3
                            hsrc, c0 = (
                                (h2e, 0) if kx == 0 else (h2o, 0) if kx == 1 else (h2e, 1)
                            )
                            nc.tensor.matmul(
                                out=ps[:, :, :, :],
                                lhsT=w3all[:, mt, 1152 * kt + 128 * g : 1152 * kt + 128 * g + 128],
                                rhs=hsrc[:, kt, :, ky : ky + 32 : 2, c0 : c0 + 16],
                                start=(n == 0),
                                stop=(n == 17),
                            )
                            n += 1
                    h3 = h3_p.tile([128, 2, 16, 16], FP, name="h3scr")
                    # accumulation runs in fp32 internally; only the final
                    # write is f32r-rounded (it feeds a tf32 matmul anyway)
                    with nc.allow_low_precision(reason="pool feeds f32r matmul"):
                        nc.scalar.activation(
                            out=h3[:, 0, :, :],
                            in_=ps[:, 0, :, :],
                            func=RELU,
                            bias=b3sb[:, mt : mt + 1],
                            accum_out=hpool[:, mt, 2 * pair : 2 * pair + 1],
                        )
                        nc.vector.tensor_scalar(
                            out=h3[:, 1, :, :], in0=ps[:, 1, :, :],
                            scalar1=b3sb[:, mt : mt + 1], scalar2=0.0,
                            op0=ADD, op1=MAX,
                        )
                        nc.vector.tensor_reduce(
                            out=hpool[:, mt, 2 * pair + 1 : 2 * pair + 2],
                            in_=h3[:, 1, :, :],
                            axis=mybir.AxisListType.XY,
                            op=ADD,
                        )
                    if fc1_hook is not None:
                        fc1_hook(mt)

            # conv3(pair p) is sandwiched between conv1 and conv2 of image
            # 2p+2: conv1's matmuls cover the latency of the last conv2
            # drains conv3 depends on, and conv3's long stretch covers
            # conv1's drains that conv2 depends on.
            fwall = None
            for img in range(8):
                pair, i = divmod(img, 2)
                h1pair = h1pads[img % 2]
                h2pair = h2pads[pair % 2]
                for half in range(2):
                    conv1_half(img, h1pair, half)
                    if half == 0 and img + 2 < 8:
                        load_xim(img + 2)
                if i == 0 and pair >= 1:
                    conv3(pair - 1, h2pads[(pair - 1) % 2])
                if img == 1:
                    fwall = consts.tile([128, 5224], BF)
                    nc.sync.dma_start(out=fwall[:, :], in_=fwall_d[:, :])
                if i == 0:
                    # defer this image's second conv2 half: it runs after the
                    # NEXT image's conv1, covering those drains' latency
                    conv2_half(img, h1pair, h2pair, i, 0)
                else:
                    conv2_half(img - 1, h1pads[(img - 1) % 2], h2pair, 0, 1)
                    conv2_half(img, h1pair, h2pair, i, 0)
                    conv2_half(img, h1pair, h2pair, i, 1)

            # FC1 with batch on partitions: psf[b, j] = fb1[j] + sum_kt
            # hpool[:, kt, b]^T @ fw1t[:, kt, j]. One 512-wide matmul per
            # (kt, half): 4 LDWEIGHTS of hpool instead of 32 of fw1t, so the
            # PE isn't weight-load-bound. Bias lands first via a K=1 matmul
            # (all-ones lhsT). Chunk kt is issued one mt-group late inside
            # conv3(pair 3) so its hpool dependency is long satisfied and
            # only kt=3 remains on the serial tail.
            psfA = ps1_p.tile([8, 512], FP, name="ps_c1")
            psfB = ps1_p.tile([8, 512], FP, name="ps_c1")
            nc.tensor.matmul(
                out=psfA[:, :], lhsT=ones[0:1, :], rhs=fwall[0:1, 4184:4696],
                start=True, stop=False,
            )
            nc.tensor.matmul(
                out=psfB[:, :], lhsT=ones[0:1, :], rhs=fwall[0:1, 4696:5208],
                start=True, stop=False,
            )

            def fc1_chunk(kt):
                nc.tensor.matmul(
                    out=psfA[:, :], lhsT=hpool[:, kt, :],
                    rhs=fwall[:, 1024 * kt : 1024 * kt + 512],
                    start=False, stop=(kt == 3),
                )
                nc.tensor.matmul(
                    out=psfB[:, :], lhsT=hpool[:, kt, :],
                    rhs=fwall[:, 1024 * kt + 512 : 1024 * kt + 1024],
                    start=False, stop=(kt == 3),
                )

            def fc1_hook(mt):
                if mt >= 1:
                    fc1_chunk(mt - 1)

            conv3(3, h2pads[1], fc1_hook=fc1_hook)
            fc1_chunk(3)

            # relu in 256-col chunks alternating engines so the first
            # transpose starts ~350ns after psfA stops instead of 1.2us
            for q in range(4):
                ps_src = psfA if q < 2 else psfB
                off = 256 * (q % 2)
                if q % 2 == 0:
                    nc.scalar.activation(
                        out=z1[:, 256 * q : 256 * q + 256],
                        in_=ps_src[:, off : off + 256], func=RELU,
                    )
                else:
                    nc.vector.tensor_scalar(
                        out=z1[:, 256 * q : 256 * q + 256],
                        in0=ps_src[:, off : off + 256],
                        scalar1=0.0, scalar2=0.0, op0=ADD, op1=MAX,
                    )

            # z1 [8, 1024] -> z1T [128, 8, 8] via PE transpose (identity rhs),
            # drained by relu (idempotent) alternating engines; FC2 accumulates
            # over the 8 column chunks.
            psf2 = ps1_p.tile([128, 8], FP, name="ps_c1")
            nc.tensor.matmul(
                out=psf2[0:10, :], lhsT=fwall[0:1, 5208:5218], rhs=ones[0:1, :],
                start=True, stop=False,
            )
            for c in range(8):
                zps = (ps23_p if c % 2 == 0 else ps3_p).tile(
                    [128, 8], FP, name="ps_c2" if c % 2 == 0 else "ps_c3"
                )
                nc.tensor.matmul(
                    out=zps[:, :], lhsT=z1[:, 128 * c : 128 * c + 128],
                    rhs=fwall[0:8, 4176:4184], start=True, stop=True,
                )
                if c % 2 == 0:
                    nc.scalar.activation(out=z1T[:, c, :], in_=zps[:, :], func=RELU)
                else:
                    nc.vector.tensor_scalar(
                        out=z1T[:, c, :], in0=zps[:, :], scalar1=0.0, scalar2=0.0,
                        op0=ADD, op1=MAX,
                    )
                nc.tensor.matmul(
                    out=psf2[0:10, :],
                    lhsT=fwall[:, 4096 + 10 * c : 4096 + 10 * c + 10],
                    rhs=z1T[:, c, :],
                    start=False,
                    stop=(c == 7),
                )
            nc.scalar.activation(out=y_sb[:, :], in_=psf2[0:10, :], func=mybir.ActivationFunctionType.Copy)
            nc.sync.dma_start(out=outT_d[:, :], in_=y_sb[:, :])

    nc.compile()
    return nc


def _get_nc():
    if "nc" not in _cache:
        _cache["nc"] = _build()
    return _cache["nc"]


def kernel(**inputs):
    from concourse import bass_utils

    nc = _get_nc()
    xpad, weights = _prep(inputs)
    in_maps = [
        dict(weights, xpad=np.ascontiguousarray(xpad[8 * c : 8 * c + 8]))
        for c in range(8)
    ]
    res = bass_utils.run_bass_kernel_spmd(
        nc, in_maps, core_ids=list(range(8)), trace=TRACE
    )
    LAST["exec_time_ns"] = getattr(res, "exec_time_ns", None)
    LAST["profile_json"] = getattr(res, "profile_json", None)
    LAST["instructions_and_trace"] = getattr(res, "instructions_and_trace", None)
    out = np.concatenate([r["outT"].T for r in res.results], axis=0)
    return np.ascontiguousarray(out.astype(np.float32))
